# revision 10
# baseline (speedup 1.0000x reference)
"""DeepSeek-MoE Trainium2 kernel (8 NeuronCores, expert-parallel), v2.

Strategy
--------
* Routing (sigmoid + grouped top-k, DeepSeek noaux_tc) is replicated on every
  core in fp32 (top-k margins in this regime are ~2e-5, so bf16 routing would
  flip expert selections).
* Dispatch/combine are dense one-hot matmuls built on-device from the routing
  result: rank-within-expert comes from an exclusive cumsum over tokens
  realized as a matmul with triangular/ones masks, and the one-hot dispatch
  matrix D[t, c] = (rank[t, e_slot] == c) is built with per-partition
  tensor_scalar(is_equal) against an iota row.
* Expert parallelism: 4 experts per core (load-balanced bin-packing computed
  on the host at call time from the actual routing), per-slot capacities are
  compile-time (multiples of 128 covering the observed loads + margin).
* Expert weights are downcast to bf16 on the host and repacked so every DMA
  reads long contiguous per-partition lines.  A deep prefetch pool on the
  gpsimd DMA queue streams w13 from t=0.
* Schedule: phase A runs all four experts' w13 GEMM + silu*up + transpose
  (actT for all capacity tiles stays in SBUF).  Phase B streams w2 one
  h-half at a time, combines routed+shared into a bf16 partial and launches
  the ReduceScatter for half 0 while half 1 is still computing.
* Shared experts are sharded over their intermediate dim (352 channels/core),
  computed in [f, t] orientation (pair-waves) so no transpose is needed.
* Cross-core combine: bf16 ReduceScatter(add) per h-half; core r returns
  tokens [64r, 64r+64) and the host concatenates the 8 slices.
"""

import numpy as np
import ml_dtypes

T, H, E, K, I = 512, 2048, 32, 8, 1408
NG, TKG = 8, 4
RSF = 2.5
NCORES = 8
P = 128
ISH = 2 * I // NCORES  # 352: shared-expert intermediate slice per core
ISHP = 384             # padded to 3*128
HT = H // P            # 16 h-tiles
TT = T // P            # 4 token tiles
IT = I // P            # 11 i-tiles
GS = E // NG           # 4 experts per group
BIG = 1.0e9

bf16 = ml_dtypes.bfloat16

# w13 f-chunks (gate|up pairs of width fw packed adjacently)
FCH = []
_fo = 0
while _fo < I:
    FCH.append((_fo, min(512, I - _fo)))
    _fo += 512
KG = 4  # ko-tiles per w13 DMA chunk
# column offset of chunk (fci, kg) in the packed w13 stream
_W13OFF = {}
_off = 0
for _fci, (_fo, _fw) in enumerate(FCH):
    for _kg in range(HT // KG):
        _W13OFF[(_fci, _kg)] = _off
        _off += KG * 2 * _fw
W13C = _off  # 45056
KOG = [(0, 3), (3, 3), (6, 3), (9, 2)]  # w2 ko groups


# ----------------------------------------------------------------------------
# Host-side routing mirror (only used to pick expert->core assignment and
# compile-time slot capacities; the device re-computes routing exactly).
# ----------------------------------------------------------------------------
def _host_loads(x, gate_w, bias):
    logits = (x.astype(np.float32) @ gate_w.astype(np.float32)).astype(np.float32)
    scores = (1.0 / (1.0 + np.exp(-logits))).astype(np.float32)
    sb = scores + bias[None, :].astype(np.float32)
    g = sb.reshape(T, NG, GS)
    pair = [g[..., i] + g[..., j] for i in range(GS) for j in range(i + 1, GS)]
    grp = np.max(np.stack(pair, -1), -1)
    gmask = np.zeros((T, NG), np.float32)
    gw = grp.copy()
    for _ in range(TKG):
        mx = gw.max(-1, keepdims=True)
        eq = (gw == mx).astype(np.float32)
        gmask += eq
        gw -= eq * BIG
    emask = np.repeat(gmask, GS, axis=1)
    m = sb + (emask * BIG - BIG)
    kmask = np.zeros((T, E), np.float32)
    for _ in range(K):
        mx = m.max(-1, keepdims=True)
        eq = (m == mx).astype(np.float32)
        kmask += eq
        m -= eq * BIG
    return kmask.sum(0)


def _plan_slots(loads, margin=2):
    caps = (np.ceil((loads + margin) / P).astype(int) * P).clip(P, None)
    order = np.argsort(-(caps * 1000 + loads))
    groups = [[] for _ in range(NCORES)]
    gsum = [0] * NCORES
    for e in order:
        cand = [i for i in sorted(range(NCORES), key=lambda i: (gsum[i], len(groups[i])))
                if len(groups[i]) < 4]
        i = cand[0]
        groups[i].append(int(e))
        gsum[i] += caps[e]
    for i in range(NCORES):
        groups[i].sort(key=lambda e: -caps[e])
    slot_caps = [int(max(caps[groups[i][j]] for i in range(NCORES))) for j in range(4)]
    return groups, slot_caps


# ----------------------------------------------------------------------------
# Device program
# ----------------------------------------------------------------------------
def _build_nc(slot_caps, single_core=False):
    import concourse.mybir as mybir
    import concourse.tile as tile
    from concourse import bacc
    from contextlib import ExitStack

    f32 = mybir.dt.float32
    b16 = mybir.dt.bfloat16
    Alu = mybir.AluOpType
    Act = mybir.ActivationFunctionType
    Ax = mybir.AxisListType

    cts = [c // P for c in slot_caps]            # ctiles per slot
    offs = np.cumsum([0] + slot_caps).tolist()   # D column offsets
    DCOLS = offs[-1]
    NCT = sum(cts)                               # total ctiles on this core
    cbase = np.cumsum([0] + cts).tolist()        # global ctile index base per slot
    CAPMAX = max(slot_caps)

    nc = bacc.Bacc("TRN2", target_bir_lowering=False, debug=False,
                   num_devices=1 if single_core else NCORES)

    # ---- I/O ----
    x_d = nc.dram_tensor("x", [T, H], f32, kind="ExternalInput")
    gw_d = nc.dram_tensor("gate_w", [H, E], f32, kind="ExternalInput")
    bias_d = nc.dram_tensor("bias_b", [P, E], f32, kind="ExternalInput")
    w13_d = nc.dram_tensor("w13p", [4, P, W13C], b16, kind="ExternalInput")
    w2_d = nc.dram_tensor("w2p", [2, 4, P, IT, 1024], b16, kind="ExternalInput")
    wgu_d = nc.dram_tensor("wgup", [3, P, HT, 256], b16, kind="ExternalInput")
    wdn_d = nc.dram_tensor("wdnp", [2, P, 3, 1024], b16, kind="ExternalInput")
    sel_d = nc.dram_tensor("sel", [E, 4], f32, kind="ExternalInput")
    iota_d = nc.dram_tensor("iota_r", [P, CAPMAX], f32, kind="ExternalInput")
    triu_d = nc.dram_tensor("triu_b", [P, P], b16, kind="ExternalInput")
    ones_d = nc.dram_tensor("ones_b", [P, P], b16, kind="ExternalInput")
    id32_d = nc.dram_tensor("id_f32", [P, P], f32, kind="ExternalInput")
    id16_d = nc.dram_tensor("id_b16", [P, P], b16, kind="ExternalInput")
    out_d = nc.dram_tensor("out_slice",
                           [T, H] if single_core else [T // NCORES, H], f32,
                           kind="ExternalOutput")

    partial_d = [nc.dram_tensor(f"partial{i}", [T, 1024], b16,
                                kind="Internal") for i in range(2)]
    rs_d = [nc.dram_tensor(f"rs_out{i}", [T // NCORES, 1024], b16,
                           kind="Internal") for i in range(2)]

    def cp(i, out, in_):
        # alternate psum/sbuf copies between DVE and ACT to balance engines
        if i % 2 == 0:
            nc.vector.tensor_copy(out=out, in_=in_)
        else:
            nc.scalar.copy(out, in_)

    xr = x_d.ap().rearrange("(tt p) h -> p tt h", p=P)
    gwr = gw_d.ap().rearrange("(ko p) e -> p ko e", p=P)

    with tile.TileContext(nc) as tc, ExitStack() as ctx:
        pc = ctx.enter_context(tc.tile_pool(name="persist", bufs=1))
        sp = ctx.enter_context(tc.tile_pool(name="smalls", bufs=2))
        psA = ctx.enter_context(tc.tile_pool(name="psumA", bufs=2, space="PSUM"))
        psB = ctx.enter_context(tc.tile_pool(name="psumB", bufs=1, space="PSUM"))
        op_ = ctx.enter_context(tc.tile_pool(name="ostage", bufs=2))
        # deep w13 prefetch stream: opened early so its DMAs start at t=0;
        # closed after phase A so phase B reuses the space
        w13ctx = ExitStack()
        w13p_pool = w13ctx.enter_context(tc.tile_pool(name="w13s", bufs=3))

        pctr = 0

        def mmw(name):
            # three rotating 2-bank wide accumulators
            nonlocal pctr
            pctr += 1
            return psB.tile([P, 1024], f32, tag=f"mmw{pctr % 3}", name=name)

        # ---- w13 stream split across two DMA queues ----
        # gpsimd half issues at t=0 (prefetch); sync half is emitted after the
        # front section so it cannot block the x / const loads on that queue.
        w13_tiles = []   # consumption order
        w13_sync = []    # (tile, j, co, ncols) deferred to the sync queue
        nchunk = 0
        for j in range(4):
            for fci, (fo, fw) in enumerate(FCH):
                for kg in range(HT // KG):
                    tg = "wga" if nchunk % 2 == 0 else "wgb"
                    wg = w13p_pool.tile([P, KG, 1024], b16, tag=tg, name="wg",
                                        bufs=3 if tg == "wga" else 2)
                    co = _W13OFF[(fci, kg)]
                    if nchunk % 2 == 0:
                        nc.gpsimd.dma_start(
                            wg.rearrange("p k f -> p (k f)")[:, :KG * 2 * fw],
                            w13_d.ap()[j, :, co:co + KG * 2 * fw])
                    else:
                        w13_sync.append((wg, j, co, KG * 2 * fw))
                    w13_tiles.append(wg)
                    nchunk += 1
        w13_tiles = iter(w13_tiles)

        # ---- constants needed first on the critical path ----
        id32_sb = pc.tile([P, P], f32, tag="id32")
        nc.sync.dma_start(id32_sb[:], id32_d.ap())
        gw_sb = pc.tile([P, HT, E], f32, tag="gw")
        nc.sync.dma_start(gw_sb[:], gwr)

        # persistent activation/data tiles
        lg_sb = pc.tile([P, TT, E], f32, tag="lg")
        actShT = pc.tile([P, 3, T], b16, tag="actShT")
        xeT = pc.tile([P, HT, DCOLS], b16, tag="xeT")
        actT = pc.tile([P, IT, NCT, P], b16, tag="actT")

        with tc.tile_pool(name="front", bufs=2) as fp:
            x_bf = fp.tile([P, TT, H], b16, tag="xb", bufs=1)
            xT_bf = fp.tile([P, HT, T], b16, tag="xTb", bufs=1)
            # ---- stream x in 512-col chunks: cast bf16, x^T (PE), logits ----
            for hc in range(4):
                xf = fp.tile([P, TT, 512], f32, tag="xf")
                nc.sync.dma_start(xf[:], xr[:, :, hc * 512:(hc + 1) * 512])
                cp(hc, x_bf[:, :, hc * 512:(hc + 1) * 512], xf[:])
                xtf = fp.tile([P, 4, T], f32, tag="xtf")  # [hp, ho_local, t]
                for hl in range(4):
                    for tt in range(TT):
                        pt = psA.tile([P, P], f32, tag="sm", name="pt_x")
                        nc.tensor.transpose(pt[:], xf[:, tt, hl * P:(hl + 1) * P],
                                            id32_sb[:])
                        cp(tt, xtf[:, hl, tt * P:(tt + 1) * P], pt[:])
                    cp(hl, xT_bf[:, hc * 4 + hl, :], xtf[:, hl, :])
                for tt in range(TT):
                    pl = psA.tile([P, E], f32, tag="sm", name="pl")
                    for hl in range(4):
                        nc.tensor.matmul(pl[:], xtf[:, hl, tt * P:(tt + 1) * P],
                                         gw_sb[:, hc * 4 + hl, :],
                                         start=(hl == 0), stop=(hl == 3))
                    if hc == 0:
                        nc.vector.tensor_copy(out=lg_sb[:, tt, :], in_=pl[:])
                    else:
                        nc.vector.tensor_tensor(lg_sb[:, tt, :], lg_sb[:, tt, :],
                                                pl[:], Alu.add)

            # ---- remaining constants (needed after logits) ----
            bias_sb = pc.tile([P, E], f32, tag="bias")
            nc.sync.dma_start(bias_sb[:], bias_d.ap())
            sel_sb = pc.tile([E, 4], f32, tag="sel")
            nc.sync.dma_start(sel_sb[:], sel_d.ap())
            iota_sb = pc.tile([P, CAPMAX], f32, tag="iota")
            nc.sync.dma_start(iota_sb[:], iota_d.ap())
            triu_sb = pc.tile([P, P], b16, tag="triu")
            nc.sync.dma_start(triu_sb[:], triu_d.ap())
            ones_sb = pc.tile([P, P], b16, tag="ones")
            nc.sync.dma_start(ones_sb[:], ones_d.ap())
            id16_sb = pc.tile([P, P], b16, tag="id16")
            nc.sync.dma_start(id16_sb[:], id16_d.ap())

            # ---- shared expert gate/up in [f, t] orientation (pair-waves) ----
            # wave w covers gate f-cols [128w,128w+128) and up f-cols likewise
            # (padded to 384); psum [f, t] holds gate|up halves in one tile.
            for w in range(3):
                psh = mmw(f"psh{w}")  # [P, 1024]: cols 0:512 gate, 512:1024 up
                for kg in range(HT // KG):
                    wguc = fp.tile([P, KG, 256], b16, tag="wguc")
                    nc.scalar.dma_start(wguc[:],
                                        wgu_d.ap()[w, :, kg * KG:(kg + 1) * KG, :])
                    for kl in range(KG):
                        ko = kg * KG + kl
                        nc.tensor.matmul(psh[:, 0:512], wguc[:, kl, 0:P],
                                         xT_bf[:, ko, :],
                                         start=(ko == 0), stop=(ko == HT - 1))
                        nc.tensor.matmul(psh[:, 512:1024], wguc[:, kl, P:256],
                                         xT_bf[:, ko, :],
                                         start=(ko == 0), stop=(ko == HT - 1))
                tmpsh = sp.tile([P, 512], b16, tag="tmpsh")
                nc.scalar.activation(tmpsh[:], psh[:, 0:512], Act.Silu)
                nc.vector.tensor_tensor(actShT[:, w, :], tmpsh[:],
                                        psh[:, 512:1024], Alu.mult)

            # ---- routing (fp32, on [P, TT, NG, GS] layouts) ----
            scores = pc.tile([P, TT, NG, GS], f32, tag="scores")
            nc.scalar.activation(scores.rearrange("p t g s -> p t (g s)"), lg_sb[:],
                                 Act.Sigmoid)
            sbb = pc.tile([P, TT, NG, GS], f32, tag="sbb")
            nc.vector.tensor_tensor(
                sbb[:], scores[:],
                bias_sb.rearrange("p (g s) -> p g s", g=NG)[:, None, :, :]
                .to_broadcast([P, TT, NG, GS]), Alu.add)

            grp = sp.tile([P, TT, NG], f32, tag="grp")
            pw = sp.tile([P, TT, NG], f32, tag="pw")
            first = True
            for i in range(GS):
                for j in range(i + 1, GS):
                    dst = grp if first else pw
                    nc.vector.tensor_tensor(dst[:], sbb[:, :, :, i], sbb[:, :, :, j],
                                            Alu.add)
                    if not first:
                        nc.vector.tensor_tensor(grp[:], grp[:], pw[:], Alu.max)
                    first = False

            gmask = sp.tile([P, TT, NG], f32, tag="gmask")
            tmpg = sp.tile([P, TT, NG], f32, tag="tmpg")
            mxg = sp.tile([P, TT], f32, tag="mxg")
            for r in range(TKG):
                nc.vector.reduce_max(mxg[:], grp[:], axis=Ax.X)
                nc.vector.tensor_tensor(tmpg[:], grp[:],
                                        mxg[:, :, None].to_broadcast([P, TT, NG]),
                                        Alu.is_equal)
                if r == 0:
                    nc.vector.tensor_copy(out=gmask[:], in_=tmpg[:])
                else:
                    nc.vector.tensor_tensor(gmask[:], gmask[:], tmpg[:], Alu.add)
                if r < TKG - 1:
                    nc.vector.tensor_scalar(tmpg[:], tmpg[:], BIG, None, Alu.mult)
                    nc.vector.tensor_tensor(grp[:], grp[:], tmpg[:], Alu.subtract)

            m_t = pc.tile([P, TT, NG, GS], f32, tag="mt")
            nc.vector.tensor_scalar(m_t[:], gmask[:, :, :, None]
                                    .to_broadcast([P, TT, NG, GS]),
                                    BIG, -BIG, Alu.mult, Alu.add)
            nc.vector.tensor_tensor(m_t[:], m_t[:], sbb[:], Alu.add)
            m_f = m_t.rearrange("p t g s -> p t (g s)")

            kmask = pc.tile([P, TT, E], f32, tag="kmask")
            tmpk = sp.tile([P, TT, E], f32, tag="tmpk")
            mxk = sp.tile([P, TT], f32, tag="mxk")
            for r in range(K):
                nc.vector.reduce_max(mxk[:], m_f, axis=Ax.X)
                nc.vector.tensor_tensor(tmpk[:], m_f,
                                        mxk[:, :, None].to_broadcast([P, TT, E]),
                                        Alu.is_equal)
                if r == 0:
                    nc.vector.tensor_copy(out=kmask[:], in_=tmpk[:])
                else:
                    nc.vector.tensor_tensor(kmask[:], kmask[:], tmpk[:], Alu.add)
                if r < K - 1:
                    nc.vector.tensor_scalar(tmpk[:], tmpk[:], BIG, None, Alu.mult)
                    nc.vector.tensor_tensor(m_f, m_f, tmpk[:], Alu.subtract)

            wsel = sp.tile([P, TT, E], f32, tag="wsel")
            nc.vector.tensor_tensor(wsel[:], kmask[:],
                                    scores.rearrange("p t g s -> p t (g s)"),
                                    Alu.mult)
            denom = sp.tile([P, TT], f32, tag="denom")
            nc.vector.reduce_sum(denom[:], wsel[:], axis=Ax.X)
            winv = sp.tile([P, TT], f32, tag="winv")
            nc.vector.reciprocal(winv[:], denom[:])
            nc.vector.tensor_scalar(winv[:], winv[:], RSF, None, Alu.mult)
            W_t = pc.tile([P, TT, E], f32, tag="Wt")
            nc.vector.tensor_tensor(W_t[:], wsel[:],
                                    winv[:, :, None].to_broadcast([P, TT, E]),
                                    Alu.mult)

            count_bf = sp.tile([P, TT, E], b16, tag="countb")
            nc.scalar.copy(count_bf[:], kmask[:])
            baseA = pc.tile([P, TT, E], f32, tag="baseA")
            namask = sp.tile([P, TT, E], f32, tag="namask")
            nc.vector.tensor_scalar(namask[:], kmask[:], -1.0e6, 1.0e6,
                                    Alu.mult, Alu.add)
            for mt in range(TT):
                pb = psA.tile([P, E], f32, tag="sm", name="pb")
                for kk in range(mt + 1):
                    lhs = ones_sb if kk < mt else triu_sb
                    nc.tensor.matmul(pb[:], lhs[:], count_bf[:, kk, :],
                                     start=(kk == 0), stop=(kk == mt))
                nc.vector.tensor_tensor(baseA[:, mt, :], pb[:], namask[:, mt, :],
                                        Alu.add)

            # transpose baseA, W -> [E, t]; select this core's 4 experts via sel
            baT = pc.tile([E, TT, P], f32, tag="baT")
            wT = pc.tile([E, TT, P], f32, tag="wT")
            for tt in range(TT):
                pt1 = psA.tile([E, P], f32, tag="sm", name="pt1")
                nc.tensor.transpose(pt1[:], baseA[:, tt, :], id32_sb[:])
                nc.vector.tensor_copy(out=baT[:, tt, :], in_=pt1[:])
                pt2 = psA.tile([E, P], f32, tag="sm", name="pt2")
                nc.tensor.transpose(pt2[:], W_t[:, tt, :], id32_sb[:])
                nc.scalar.copy(wT[:, tt, :], pt2[:])
            bsel = pc.tile([P, TT, 4], f32, tag="bsel")
            wsel4 = pc.tile([P, TT, 4], f32, tag="wsel4")
            for tt in range(TT):
                pb4 = psA.tile([P, 4], f32, tag="sm", name="pb4")
                nc.tensor.matmul(pb4[:], baT[:, tt, :], sel_sb[:], start=True,
                                 stop=True)
                nc.vector.tensor_copy(out=bsel[:, tt, :], in_=pb4[:])
                pw4 = psA.tile([P, 4], f32, tag="sm", name="pw4")
                nc.tensor.matmul(pw4[:], wT[:, tt, :], sel_sb[:], start=True,
                                 stop=True)
                nc.scalar.copy(wsel4[:, tt, :], pw4[:])

            # dispatch one-hot D (bf16); combine weights Wc (bf16) -> WcT
            D_sb = pc.tile([P, TT, DCOLS], b16, tag="D")
            WcT = pc.tile([P, NCT, T], b16, tag="WcT")
            for tt in range(TT):
                for j in range(4):
                    cap = slot_caps[j]
                    nc.vector.tensor_scalar(D_sb[:, tt, offs[j]:offs[j] + cap],
                                            iota_sb[:, :cap], bsel[:, tt, j:j + 1],
                                            None, Alu.is_equal)
                    wcs = sp.tile([P, 256], b16, tag="wcs")
                    nc.vector.tensor_scalar(wcs[:, :cap], iota_sb[:, :cap],
                                            bsel[:, tt, j:j + 1],
                                            wsel4[:, tt, j:j + 1],
                                            Alu.is_equal, Alu.mult)
                    for cl in range(cts[j]):
                        ptw = psA.tile([P, P], b16, tag="sm", name="ptw")
                        nc.tensor.transpose(ptw[:], wcs[:, cl * P:(cl + 1) * P],
                                            id16_sb[:])
                        cp(cl + tt, WcT[:, cbase[j] + cl, tt * P:(tt + 1) * P],
                           ptw[:])

            # ---- dispatch matmul: xeT[h, c] = sum_t x[t,h] D[t,c] ----
            for ko in range(HT):
                px = mmw("px")
                for tt in range(TT):
                    for q0 in range(0, DCOLS, 512):
                        qw = min(512, DCOLS - q0)
                        nc.tensor.matmul(
                            px[:, q0:q0 + qw],
                            x_bf[:, tt, ko * P:(ko + 1) * P],
                            D_sb[:, tt, q0:q0 + qw],
                            start=(tt == 0), stop=(tt == TT - 1))
                cp(ko, xeT[:, ko, :], px[:, :DCOLS])
        # front pool released here

        # deferred sync-queue half of the w13 stream (emitted after the front
        # section so the x/const loads on the sync queue run first)
        for (wg, j, co, ncols) in w13_sync:
            nc.sync.dma_start(wg.rearrange("p k f -> p (k f)")[:, :ncols],
                              w13_d.ap()[j, :, co:co + ncols])


        # ---- phase A: all slots w13 -> act -> actT ----
        with tc.tile_pool(name="expA", bufs=2) as ea:
            for j in range(4):
                ct = cts[j]
                act = ea.tile([P, 2, I], b16, tag="act", name="act")
                for fci, (fo, fw) in enumerate(FCH):
                    pgus = [mmw(f"pgu{ci}") for ci in range(ct)]
                    for kg in range(HT // KG):
                        wg = next(w13_tiles)
                        for kl in range(KG):
                            ko = kg * KG + kl
                            for ci in range(ct):
                                lhs = xeT[:, ko,
                                          offs[j] + ci * P: offs[j] + (ci + 1) * P]
                                for q0 in range(0, 2 * fw, 512):
                                    qw = min(512, 2 * fw - q0)
                                    nc.tensor.matmul(
                                        pgus[ci][:, q0:q0 + qw], lhs,
                                        wg.rearrange("p k f -> p (k f)")
                                        [:, kl * 2 * fw + q0:kl * 2 * fw + q0 + qw],
                                        start=(ko == 0),
                                        stop=(ko == HT - 1))
                    for ci in range(ct):
                        tmpa = sp.tile([P, 512], b16, tag="tmpa")
                        nc.scalar.activation(tmpa[:, :fw], pgus[ci][:, :fw],
                                             Act.Silu)
                        nc.vector.tensor_tensor(act[:, ci, fo:fo + fw],
                                                tmpa[:, :fw],
                                                pgus[ci][:, fw:2 * fw],
                                                Alu.mult)
                # transpose act -> actT [i, ctile]
                for ci in range(ct):
                    for io in range(IT):
                        pt4 = psA.tile([P, P], b16, tag="sm", name="pt4")
                        nc.tensor.transpose(pt4[:], act[:, ci, io * P:(io + 1) * P],
                                            id16_sb[:])
                        cp(io, actT[:, io, cbase[j] + ci, :], pt4[:])

        w13ctx.close()  # release w13 stream space before phase B

        # ---- phase B: w2 per h-half, combine, overlapped ReduceScatter ----
        with tc.tile_pool(name="phB", bufs=3) as pb_:
            yes = [pb_.tile([P, 1024], b16, tag=f"ye{cb}", name=f"ye{cb}",
                            bufs=1) for cb in range(NCT)]
            # preload shared-down weights for BOTH h-halves before any
            # collective is issued (a DMA behind an in-flight collective can
            # stall on some queues).
            wdn_sb = pb_.tile([P, 2, 3, 1024], b16, tag="wdn", bufs=1)
            nc.scalar.dma_start(wdn_sb[:, 0], wdn_d.ap()[0])
            nc.scalar.dma_start(wdn_sb[:, 1], wdn_d.ap()[1])
            for hh in range(2):
                for j in range(4):
                    ct = cts[j]
                    pys = [mmw(f"py{ci}") for ci in range(ct)]
                    for kgi, (ko0, kn) in enumerate(KOG):
                        par = (hh * 4 + j + kgi) % 2
                        w2c = pb_.tile([P, 3, 1024], b16,
                                       tag="w2ca" if par == 0 else "w2cb",
                                       name="w2c", bufs=3)
                        eng2 = nc.sync if par == 0 else nc.scalar
                        eng2.dma_start(w2c[:, :kn, :],
                                       w2_d.ap()[hh, j, :, ko0:ko0 + kn, :])
                        for kl in range(kn):
                            ko = ko0 + kl
                            for ci in range(ct):
                                for q0 in (0, 512):
                                    nc.tensor.matmul(
                                        pys[ci][:, q0:q0 + 512],
                                        actT[:, ko, cbase[j] + ci, :],
                                        w2c[:, kl, q0:q0 + 512],
                                        start=(ko == 0), stop=(ko == IT - 1))
                    for ci in range(ct):
                        nc.vector.tensor_copy(out=yes[cbase[j] + ci][:],
                                              in_=pys[ci][:])

                # combine: routed ctiles + shared slice -> bf16 partial
                for tt in range(TT):
                    po = mmw("po")
                    for q0 in (0, 512):
                        for q, cb in enumerate(range(NCT)):
                            nc.tensor.matmul(
                                po[:, q0:q0 + 512],
                                WcT[:, cb, tt * P:(tt + 1) * P],
                                yes[cb][:, q0:q0 + 512],
                                start=(q == 0), stop=False)
                        for io in range(3):
                            nc.tensor.matmul(
                                po[:, q0:q0 + 512],
                                actShT[:, io, tt * P:(tt + 1) * P],
                                wdn_sb[:, hh, io, q0:q0 + 512],
                                start=False, stop=(io == 2))
                    if single_core:
                        stg32 = op_.tile([P, 1024], f32, tag="stg32")
                        nc.vector.tensor_copy(out=stg32[:], in_=po[:])
                        nc.sync.dma_start(
                            out_d.ap()[tt * P:(tt + 1) * P,
                                       hh * 1024:(hh + 1) * 1024], stg32[:])
                    else:
                        stg = op_.tile([P, 1024], b16, tag="stg")
                        nc.vector.tensor_copy(out=stg[:], in_=po[:])
                        nc.sync.dma_start(
                            partial_d[hh].ap()[tt * P:(tt + 1) * P, :], stg[:])

                if not single_core:
                    nc.gpsimd.collective_compute(
                        "ReduceScatter", Alu.add,
                        replica_groups=[list(range(NCORES))],
                        ins=[partial_d[hh].ap().opt()],
                        outs=[rs_d[hh].ap().opt()],
                    )

            # epilogue: rs (bf16) -> fp32 out slice
            if not single_core:
                for hh in range(2):
                    rs_sb = op_.tile([T // NCORES, 1024], b16, tag="rs_sb")
                    nc.sync.dma_start(rs_sb[:], rs_d[hh].ap())
                    rs_f = op_.tile([T // NCORES, 1024], f32, tag="rs_f")
                    nc.vector.tensor_copy(out=rs_f[:], in_=rs_sb[:])
                    nc.sync.dma_start(
                        out_d.ap()[:, hh * 1024:(hh + 1) * 1024], rs_f[:])

    nc.compile()
    return nc


_NC_CACHE = {}


def _pack_inputs(x, gate_w, bias, w13, w2, sgu, sdn, groups, slot_caps):
    """Per-core in_maps with DMA-friendly packed weight layouts."""
    CAPMAX = max(slot_caps)
    iota = np.tile(np.arange(CAPMAX, dtype=np.float32), (P, 1))
    triu = np.triu(np.ones((P, P), np.float32), 1).astype(bf16)
    ones = np.ones((P, P), bf16)
    id32 = np.eye(P, dtype=np.float32)
    id16 = np.eye(P, dtype=np.float32).astype(bf16)
    bias_b = np.tile(bias[None, :], (P, 1)).astype(np.float32)

    def pack_w13(w):   # w: [H, 2I] fp32 -> packed [P, W13C] bf16
        blocks = []
        for fo, fw in FCH:
            for kg in range(HT // KG):
                for kl in range(KG):
                    ko = kg * KG + kl
                    rows = slice(ko * P, (ko + 1) * P)
                    blocks.append(np.concatenate(
                        [w[rows, fo:fo + fw], w[rows, I + fo:I + fo + fw]],
                        axis=1))
        return np.concatenate(blocks, axis=1).astype(bf16)

    def pack_w2(w):    # w: [I, H] fp32 -> [2, P, IT, 1024] bf16
        r = w.reshape(IT, P, H).transpose(1, 0, 2)  # [P, IT, H]
        return np.stack([r[:, :, 0:1024], r[:, :, 1024:2048]]).astype(bf16)

    in_maps = []
    for core in range(NCORES):
        sel = np.zeros((E, 4), np.float32)
        for j, e in enumerate(groups[core]):
            sel[e, j] = 1.0
        # shared slices, padded to 384
        gate = np.zeros((H, ISHP), np.float32)
        up = np.zeros((H, ISHP), np.float32)
        gate[:, :ISH] = sgu[:, core * ISH:(core + 1) * ISH]
        up[:, :ISH] = sgu[:, 2 * I + core * ISH:2 * I + (core + 1) * ISH]
        wgup = np.zeros((3, P, HT, 256), np.float32)
        for w in range(3):
            pairc = np.concatenate(
                [gate[:, w * P:(w + 1) * P], up[:, w * P:(w + 1) * P]], axis=1)
            wgup[w] = pairc.reshape(HT, P, 256).transpose(1, 0, 2)
        dn = np.zeros((ISHP, H), np.float32)
        dn[:ISH] = sdn[core * ISH:(core + 1) * ISH, :]
        dnr = dn.reshape(3, P, H).transpose(1, 0, 2)  # [P, 3, H]
        wdnp = np.stack([dnr[:, :, 0:1024], dnr[:, :, 1024:2048]])

        in_maps.append({
            "x": x, "gate_w": gate_w, "bias_b": bias_b,
            "w13p": np.stack([pack_w13(w13[e]) for e in groups[core]]),
            "w2p": np.stack([pack_w2(w2[e]) for e in groups[core]], axis=1),
            "wgup": np.ascontiguousarray(wgup.astype(bf16)),
            "wdnp": np.ascontiguousarray(wdnp.astype(bf16)),
            "sel": sel, "iota_r": iota, "triu_b": triu, "ones_b": ones,
            "id_f32": id32, "id_b16": id16,
        })
    return in_maps


def kernel(hidden_states, residual, gate_w, bias, w13, w2, shared_gate_up,
           shared_down):
    from concourse.bass_utils import run_bass_kernel_spmd

    x = np.ascontiguousarray(np.asarray(hidden_states, np.float32))
    gate_w = np.ascontiguousarray(np.asarray(gate_w, np.float32))
    bias = np.asarray(bias, np.float32)
    w13 = np.asarray(w13, np.float32)
    w2 = np.asarray(w2, np.float32)
    sgu = np.asarray(shared_gate_up, np.float32)
    sdn = np.asarray(shared_down, np.float32)

    loads = _host_loads(x, gate_w, bias)
    groups, slot_caps = _plan_slots(loads)

    key = tuple(slot_caps)
    if key not in _NC_CACHE:
        _NC_CACHE[key] = _build_nc(slot_caps)
    nc = _NC_CACHE[key]

    in_maps = _pack_inputs(x, gate_w, bias, w13, w2, sgu, sdn, groups,
                           slot_caps)
    res = run_bass_kernel_spmd(nc, in_maps, core_ids=list(range(NCORES)))
    out = np.concatenate([res.results[c]["out_slice"] for c in range(NCORES)],
                         axis=0)
    return out.astype(np.float32)


# revision 11
# speedup vs baseline: 1.0200x; 1.0200x over previous
"""DeepSeek-MoE Trainium2 kernel (8 NeuronCores, expert-parallel), v2.

Strategy
--------
* Routing (sigmoid + grouped top-k, DeepSeek noaux_tc) is replicated on every
  core in fp32 (top-k margins in this regime are ~2e-5, so bf16 routing would
  flip expert selections).
* Dispatch/combine are dense one-hot matmuls built on-device from the routing
  result: rank-within-expert comes from an exclusive cumsum over tokens
  realized as a matmul with triangular/ones masks, and the one-hot dispatch
  matrix D[t, c] = (rank[t, e_slot] == c) is built with per-partition
  tensor_scalar(is_equal) against an iota row.
* Expert parallelism: 4 experts per core (load-balanced bin-packing computed
  on the host at call time from the actual routing), per-slot capacities are
  compile-time (multiples of 128 covering the observed loads + margin).
* Expert weights are downcast to bf16 on the host and repacked so every DMA
  reads long contiguous per-partition lines.  A deep prefetch pool on the
  gpsimd DMA queue streams w13 from t=0.
* Schedule: phase A runs all four experts' w13 GEMM + silu*up + transpose
  (actT for all capacity tiles stays in SBUF).  Phase B streams w2 one
  h-half at a time, combines routed+shared into a bf16 partial and launches
  the ReduceScatter for half 0 while half 1 is still computing.
* Shared experts are sharded over their intermediate dim (352 channels/core),
  computed in [f, t] orientation (pair-waves) so no transpose is needed.
* Cross-core combine: bf16 ReduceScatter(add) per h-half; core r returns
  tokens [64r, 64r+64) and the host concatenates the 8 slices.
"""

import numpy as np
import ml_dtypes

T, H, E, K, I = 512, 2048, 32, 8, 1408
NG, TKG = 8, 4
RSF = 2.5
NCORES = 8
P = 128
ISH = 2 * I // NCORES  # 352: shared-expert intermediate slice per core
ISHP = 384             # padded to 3*128
HT = H // P            # 16 h-tiles
TT = T // P            # 4 token tiles
IT = I // P            # 11 i-tiles
GS = E // NG           # 4 experts per group
BIG = 1.0e9

bf16 = ml_dtypes.bfloat16

# w13 f-chunks (gate|up pairs of width fw packed adjacently)
FCH = []
_fo = 0
while _fo < I:
    FCH.append((_fo, min(512, I - _fo)))
    _fo += 512
KG = 4  # ko-tiles per w13 DMA chunk
# column offset of chunk (fci, kg) in the packed w13 stream
_W13OFF = {}
_off = 0
for _fci, (_fo, _fw) in enumerate(FCH):
    for _kg in range(HT // KG):
        _W13OFF[(_fci, _kg)] = _off
        _off += KG * 2 * _fw
W13C = _off  # 45056
KOG = [(0, 3), (3, 3), (6, 3), (9, 2)]  # w2 ko groups


# ----------------------------------------------------------------------------
# Host-side routing mirror (only used to pick expert->core assignment and
# compile-time slot capacities; the device re-computes routing exactly).
# ----------------------------------------------------------------------------
def _host_loads(x, gate_w, bias):
    logits = (x.astype(np.float32) @ gate_w.astype(np.float32)).astype(np.float32)
    scores = (1.0 / (1.0 + np.exp(-logits))).astype(np.float32)
    sb = scores + bias[None, :].astype(np.float32)
    g = sb.reshape(T, NG, GS)
    pair = [g[..., i] + g[..., j] for i in range(GS) for j in range(i + 1, GS)]
    grp = np.max(np.stack(pair, -1), -1)
    gmask = np.zeros((T, NG), np.float32)
    gw = grp.copy()
    for _ in range(TKG):
        mx = gw.max(-1, keepdims=True)
        eq = (gw == mx).astype(np.float32)
        gmask += eq
        gw -= eq * BIG
    emask = np.repeat(gmask, GS, axis=1)
    m = sb + (emask * BIG - BIG)
    kmask = np.zeros((T, E), np.float32)
    for _ in range(K):
        mx = m.max(-1, keepdims=True)
        eq = (m == mx).astype(np.float32)
        kmask += eq
        m -= eq * BIG
    return kmask.sum(0)


def _plan_slots(loads, margin=2):
    caps = (np.ceil((loads + margin) / P).astype(int) * P).clip(P, None)
    order = np.argsort(-(caps * 1000 + loads))
    groups = [[] for _ in range(NCORES)]
    gsum = [0] * NCORES
    for e in order:
        cand = [i for i in sorted(range(NCORES), key=lambda i: (gsum[i], len(groups[i])))
                if len(groups[i]) < 4]
        i = cand[0]
        groups[i].append(int(e))
        gsum[i] += caps[e]
    for i in range(NCORES):
        groups[i].sort(key=lambda e: -caps[e])
    slot_caps = [int(max(caps[groups[i][j]] for i in range(NCORES))) for j in range(4)]
    return groups, slot_caps


# ----------------------------------------------------------------------------
# Device program
# ----------------------------------------------------------------------------
def _build_nc(slot_caps, single_core=False):
    import concourse.mybir as mybir
    import concourse.tile as tile
    from concourse import bacc
    from contextlib import ExitStack

    f32 = mybir.dt.float32
    b16 = mybir.dt.bfloat16
    Alu = mybir.AluOpType
    Act = mybir.ActivationFunctionType
    Ax = mybir.AxisListType

    cts = [c // P for c in slot_caps]            # ctiles per slot
    offs = np.cumsum([0] + slot_caps).tolist()   # D column offsets
    DCOLS = offs[-1]
    NCT = sum(cts)                               # total ctiles on this core
    cbase = np.cumsum([0] + cts).tolist()        # global ctile index base per slot
    CAPMAX = max(slot_caps)

    nc = bacc.Bacc("TRN2", target_bir_lowering=False, debug=False,
                   num_devices=1 if single_core else NCORES)

    # ---- I/O ----
    x_d = nc.dram_tensor("x", [T, H], f32, kind="ExternalInput")
    gw_d = nc.dram_tensor("gate_w", [H, E], f32, kind="ExternalInput")
    bias_d = nc.dram_tensor("bias_b", [P, E], f32, kind="ExternalInput")
    w13_d = nc.dram_tensor("w13p", [4, P, W13C], b16, kind="ExternalInput")
    w2_d = nc.dram_tensor("w2p", [2, 4, P, IT, 1024], b16, kind="ExternalInput")
    wgu_d = nc.dram_tensor("wgup", [3, P, HT, 256], b16, kind="ExternalInput")
    wdn_d = nc.dram_tensor("wdnp", [2, P, 3, 1024], b16, kind="ExternalInput")
    sel_d = nc.dram_tensor("sel", [E, 4], f32, kind="ExternalInput")
    iota_d = nc.dram_tensor("iota_r", [P, CAPMAX], f32, kind="ExternalInput")
    triu_d = nc.dram_tensor("triu_b", [P, P], b16, kind="ExternalInput")
    ones_d = nc.dram_tensor("ones_b", [P, P], b16, kind="ExternalInput")
    id32_d = nc.dram_tensor("id_f32", [P, P], f32, kind="ExternalInput")
    id16_d = nc.dram_tensor("id_b16", [P, P], b16, kind="ExternalInput")
    out_d = nc.dram_tensor("out_slice",
                           [T, H] if single_core else [T // NCORES, H], f32,
                           kind="ExternalOutput")

    partial_d = [nc.dram_tensor(f"partial{i}", [T, 1024], b16,
                                kind="Internal") for i in range(2)]
    rs_d = [nc.dram_tensor(f"rs_out{i}", [T // NCORES, 1024], b16,
                           kind="Internal") for i in range(2)]

    def cp(i, out, in_):
        # alternate psum/sbuf copies between DVE and ACT to balance engines
        if i % 2 == 0:
            nc.vector.tensor_copy(out=out, in_=in_)
        else:
            nc.scalar.copy(out, in_)

    xr = x_d.ap().rearrange("(tt p) h -> p tt h", p=P)
    gwr = gw_d.ap().rearrange("(ko p) e -> p ko e", p=P)

    with tile.TileContext(nc) as tc, ExitStack() as ctx:
        pc = ctx.enter_context(tc.tile_pool(name="persist", bufs=1))
        sp = ctx.enter_context(tc.tile_pool(name="smalls", bufs=2))
        psA = ctx.enter_context(tc.tile_pool(name="psumA", bufs=2, space="PSUM"))
        psB = ctx.enter_context(tc.tile_pool(name="psumB", bufs=1, space="PSUM"))
        op_ = ctx.enter_context(tc.tile_pool(name="ostage", bufs=2))
        # deep w13 prefetch stream: opened early so its DMAs start at t=0;
        # closed after phase A so phase B reuses the space
        w13ctx = ExitStack()
        w13p_pool = w13ctx.enter_context(tc.tile_pool(name="w13s", bufs=3))

        pctr = 0

        def mmw(name):
            # three rotating 2-bank wide accumulators
            nonlocal pctr
            pctr += 1
            return psB.tile([P, 1024], f32, tag=f"mmw{pctr % 3}", name=name)

        # ---- w13 stream split across two DMA queues ----
        # gpsimd half issues at t=0 (prefetch); sync half is emitted after the
        # front section so it cannot block the x / const loads on that queue.
        w13_tiles = []   # consumption order
        w13_sync = []    # (tile, j, co, ncols) deferred to the sync queue
        nchunk = 0
        for j in range(4):
            for fci, (fo, fw) in enumerate(FCH):
                for kg in range(HT // KG):
                    tg = "wga" if nchunk % 2 == 0 else "wgb"
                    wg = w13p_pool.tile([P, KG, 1024], b16, tag=tg, name="wg",
                                        bufs=3 if tg == "wga" else 2)
                    co = _W13OFF[(fci, kg)]
                    if nchunk % 2 == 0:
                        nc.gpsimd.dma_start(
                            wg.rearrange("p k f -> p (k f)")[:, :KG * 2 * fw],
                            w13_d.ap()[j, :, co:co + KG * 2 * fw])
                    else:
                        w13_sync.append((wg, j, co, KG * 2 * fw))
                    w13_tiles.append(wg)
                    nchunk += 1
        w13_tiles = iter(w13_tiles)

        # ---- constants needed first on the critical path ----
        id32_sb = pc.tile([P, P], f32, tag="id32")
        nc.sync.dma_start(id32_sb[:], id32_d.ap())
        gw_sb = pc.tile([P, HT, E], f32, tag="gw")
        nc.sync.dma_start(gw_sb[:], gwr)

        # persistent activation/data tiles
        lg_sb = pc.tile([P, TT, E], f32, tag="lg")
        actShT = pc.tile([P, 3, T], b16, tag="actShT")
        xeT = pc.tile([P, HT, DCOLS], b16, tag="xeT")
        actT = pc.tile([P, IT, NCT, P], b16, tag="actT")

        with tc.tile_pool(name="front", bufs=2) as fp:
            x_bf = fp.tile([P, TT, H], b16, tag="xb", bufs=1)
            xT_bf = fp.tile([P, HT, T], b16, tag="xTb", bufs=1)
            # ---- stream x in 512-col chunks: cast bf16, x^T (PE), logits ----
            for hc in range(4):
                xf = fp.tile([P, TT, 512], f32, tag="xf")
                nc.sync.dma_start(xf[:], xr[:, :, hc * 512:(hc + 1) * 512])
                cp(hc, x_bf[:, :, hc * 512:(hc + 1) * 512], xf[:])
                xtf = fp.tile([P, 4, T], f32, tag="xtf")  # [hp, ho_local, t]
                for hl in range(4):
                    for tt in range(TT):
                        pt = psA.tile([P, P], f32, tag="sm", name="pt_x")
                        nc.tensor.transpose(pt[:], xf[:, tt, hl * P:(hl + 1) * P],
                                            id32_sb[:])
                        cp(tt, xtf[:, hl, tt * P:(tt + 1) * P], pt[:])
                    cp(hl, xT_bf[:, hc * 4 + hl, :], xtf[:, hl, :])
                for tt in range(TT):
                    pl = psA.tile([P, E], f32, tag="sm", name="pl")
                    for hl in range(4):
                        nc.tensor.matmul(pl[:], xtf[:, hl, tt * P:(tt + 1) * P],
                                         gw_sb[:, hc * 4 + hl, :],
                                         start=(hl == 0), stop=(hl == 3))
                    if hc == 0:
                        nc.vector.tensor_copy(out=lg_sb[:, tt, :], in_=pl[:])
                    else:
                        nc.vector.tensor_tensor(lg_sb[:, tt, :], lg_sb[:, tt, :],
                                                pl[:], Alu.add)

            # ---- remaining constants (needed after logits) ----
            bias_sb = pc.tile([P, E], f32, tag="bias")
            nc.sync.dma_start(bias_sb[:], bias_d.ap())
            sel_sb = pc.tile([E, 4], f32, tag="sel")
            nc.sync.dma_start(sel_sb[:], sel_d.ap())
            iota_sb = pc.tile([P, CAPMAX], f32, tag="iota")
            nc.sync.dma_start(iota_sb[:], iota_d.ap())
            triu_sb = pc.tile([P, P], b16, tag="triu")
            nc.sync.dma_start(triu_sb[:], triu_d.ap())
            ones_sb = pc.tile([P, P], b16, tag="ones")
            nc.sync.dma_start(ones_sb[:], ones_d.ap())
            id16_sb = pc.tile([P, P], b16, tag="id16")
            nc.sync.dma_start(id16_sb[:], id16_d.ap())

            # ---- shared expert gate/up in [f, t] orientation (pair-waves) ----
            # wave w covers gate f-cols [128w,128w+128) and up f-cols likewise
            # (padded to 384); psum [f, t] holds gate|up halves in one tile.
            for w in range(3):
                psh = mmw(f"psh{w}")  # [P, 1024]: cols 0:512 gate, 512:1024 up
                for kg in range(HT // KG):
                    wguc = fp.tile([P, KG, 256], b16, tag="wguc")
                    nc.scalar.dma_start(wguc[:],
                                        wgu_d.ap()[w, :, kg * KG:(kg + 1) * KG, :])
                    for kl in range(KG):
                        ko = kg * KG + kl
                        nc.tensor.matmul(psh[:, 0:512], wguc[:, kl, 0:P],
                                         xT_bf[:, ko, :],
                                         start=(ko == 0), stop=(ko == HT - 1))
                        nc.tensor.matmul(psh[:, 512:1024], wguc[:, kl, P:256],
                                         xT_bf[:, ko, :],
                                         start=(ko == 0), stop=(ko == HT - 1))
                tmpsh = sp.tile([P, 512], b16, tag="tmpsh")
                nc.scalar.activation(tmpsh[:], psh[:, 0:512], Act.Silu)
                nc.vector.tensor_tensor(actShT[:, w, :], tmpsh[:],
                                        psh[:, 512:1024], Alu.mult)

            # ---- routing (fp32, on [P, TT, NG, GS] layouts) ----
            scores = pc.tile([P, TT, NG, GS], f32, tag="scores")
            nc.scalar.activation(scores.rearrange("p t g s -> p t (g s)"), lg_sb[:],
                                 Act.Sigmoid)
            sbb = pc.tile([P, TT, NG, GS], f32, tag="sbb")
            nc.vector.tensor_tensor(
                sbb[:], scores[:],
                bias_sb.rearrange("p (g s) -> p g s", g=NG)[:, None, :, :]
                .to_broadcast([P, TT, NG, GS]), Alu.add)

            grp = sp.tile([P, TT, NG], f32, tag="grp")
            pw = sp.tile([P, TT, NG], f32, tag="pw")
            first = True
            for i in range(GS):
                for j in range(i + 1, GS):
                    dst = grp if first else pw
                    nc.vector.tensor_tensor(dst[:], sbb[:, :, :, i], sbb[:, :, :, j],
                                            Alu.add)
                    if not first:
                        nc.vector.tensor_tensor(grp[:], grp[:], pw[:], Alu.max)
                    first = False

            gmask = sp.tile([P, TT, NG], f32, tag="gmask")
            tmpg = sp.tile([P, TT, NG], f32, tag="tmpg")
            mxg = sp.tile([P, TT], f32, tag="mxg")
            for r in range(TKG):
                nc.vector.reduce_max(mxg[:], grp[:], axis=Ax.X)
                nc.vector.tensor_tensor(tmpg[:], grp[:],
                                        mxg[:, :, None].to_broadcast([P, TT, NG]),
                                        Alu.is_equal)
                if r == 0:
                    nc.vector.tensor_copy(out=gmask[:], in_=tmpg[:])
                else:
                    nc.vector.tensor_tensor(gmask[:], gmask[:], tmpg[:], Alu.add)
                if r < TKG - 1:
                    nc.vector.tensor_scalar(tmpg[:], tmpg[:], BIG, None, Alu.mult)
                    nc.vector.tensor_tensor(grp[:], grp[:], tmpg[:], Alu.subtract)

            m_t = pc.tile([P, TT, NG, GS], f32, tag="mt")
            nc.vector.tensor_scalar(m_t[:], gmask[:, :, :, None]
                                    .to_broadcast([P, TT, NG, GS]),
                                    BIG, -BIG, Alu.mult, Alu.add)
            nc.vector.tensor_tensor(m_t[:], m_t[:], sbb[:], Alu.add)
            m_f = m_t.rearrange("p t g s -> p t (g s)")

            kmask = pc.tile([P, TT, E], f32, tag="kmask")
            tmpk = sp.tile([P, TT, E], f32, tag="tmpk")
            mxk = sp.tile([P, TT], f32, tag="mxk")
            for r in range(K):
                nc.vector.reduce_max(mxk[:], m_f, axis=Ax.X)
                nc.vector.tensor_tensor(tmpk[:], m_f,
                                        mxk[:, :, None].to_broadcast([P, TT, E]),
                                        Alu.is_equal)
                if r == 0:
                    nc.vector.tensor_copy(out=kmask[:], in_=tmpk[:])
                else:
                    nc.vector.tensor_tensor(kmask[:], kmask[:], tmpk[:], Alu.add)
                if r < K - 1:
                    nc.vector.tensor_scalar(tmpk[:], tmpk[:], BIG, None, Alu.mult)
                    nc.vector.tensor_tensor(m_f, m_f, tmpk[:], Alu.subtract)

            wsel = sp.tile([P, TT, E], f32, tag="wsel")
            nc.vector.tensor_tensor(wsel[:], kmask[:],
                                    scores.rearrange("p t g s -> p t (g s)"),
                                    Alu.mult)
            denom = sp.tile([P, TT], f32, tag="denom")
            nc.vector.reduce_sum(denom[:], wsel[:], axis=Ax.X)
            winv = sp.tile([P, TT], f32, tag="winv")
            nc.vector.reciprocal(winv[:], denom[:])
            nc.vector.tensor_scalar(winv[:], winv[:], RSF, None, Alu.mult)
            W_t = pc.tile([P, TT, E], f32, tag="Wt")
            nc.vector.tensor_tensor(W_t[:], wsel[:],
                                    winv[:, :, None].to_broadcast([P, TT, E]),
                                    Alu.mult)

            count_bf = sp.tile([P, TT, E], b16, tag="countb")
            nc.scalar.copy(count_bf[:], kmask[:])
            baseA = pc.tile([P, TT, E], f32, tag="baseA")
            namask = sp.tile([P, TT, E], f32, tag="namask")
            nc.vector.tensor_scalar(namask[:], kmask[:], -1.0e6, 1.0e6,
                                    Alu.mult, Alu.add)
            for mt in range(TT):
                pb = psA.tile([P, E], f32, tag="sm", name="pb")
                for kk in range(mt + 1):
                    lhs = ones_sb if kk < mt else triu_sb
                    nc.tensor.matmul(pb[:], lhs[:], count_bf[:, kk, :],
                                     start=(kk == 0), stop=(kk == mt))
                nc.vector.tensor_tensor(baseA[:, mt, :], pb[:], namask[:, mt, :],
                                        Alu.add)

            # transpose baseA, W -> [E, t]; select this core's 4 experts via sel
            baT = pc.tile([E, TT, P], f32, tag="baT")
            wT = pc.tile([E, TT, P], f32, tag="wT")
            for tt in range(TT):
                pt1 = psA.tile([E, P], f32, tag="sm", name="pt1")
                nc.tensor.transpose(pt1[:], baseA[:, tt, :], id32_sb[:])
                nc.vector.tensor_copy(out=baT[:, tt, :], in_=pt1[:])
                pt2 = psA.tile([E, P], f32, tag="sm", name="pt2")
                nc.tensor.transpose(pt2[:], W_t[:, tt, :], id32_sb[:])
                nc.scalar.copy(wT[:, tt, :], pt2[:])
            bsel = pc.tile([P, TT, 4], f32, tag="bsel")
            wsel4 = pc.tile([P, TT, 4], f32, tag="wsel4")
            for tt in range(TT):
                pb4 = psA.tile([P, 4], f32, tag="sm", name="pb4")
                nc.tensor.matmul(pb4[:], baT[:, tt, :], sel_sb[:], start=True,
                                 stop=True)
                nc.vector.tensor_copy(out=bsel[:, tt, :], in_=pb4[:])
                pw4 = psA.tile([P, 4], f32, tag="sm", name="pw4")
                nc.tensor.matmul(pw4[:], wT[:, tt, :], sel_sb[:], start=True,
                                 stop=True)
                nc.scalar.copy(wsel4[:, tt, :], pw4[:])

            # dispatch one-hot D (bf16); combine weights Wc (bf16) -> WcT
            D_sb = pc.tile([P, TT, DCOLS], b16, tag="D")
            WcT = pc.tile([P, NCT, T], b16, tag="WcT")
            for tt in range(TT):
                for j in range(4):
                    cap = slot_caps[j]
                    nc.vector.tensor_scalar(D_sb[:, tt, offs[j]:offs[j] + cap],
                                            iota_sb[:, :cap], bsel[:, tt, j:j + 1],
                                            None, Alu.is_equal)
                    wcs = sp.tile([P, 256], b16, tag="wcs")
                    nc.vector.tensor_scalar(wcs[:, :cap], iota_sb[:, :cap],
                                            bsel[:, tt, j:j + 1],
                                            wsel4[:, tt, j:j + 1],
                                            Alu.is_equal, Alu.mult)
                    for cl in range(cts[j]):
                        ptw = psA.tile([P, P], b16, tag="sm", name="ptw")
                        nc.tensor.transpose(ptw[:], wcs[:, cl * P:(cl + 1) * P],
                                            id16_sb[:])
                        cp(cl + tt, WcT[:, cbase[j] + cl, tt * P:(tt + 1) * P],
                           ptw[:])

            # ---- dispatch matmul: xeT[h, c] = sum_t x[t,h] D[t,c] ----
            for ko in range(HT):
                px = mmw("px")
                for tt in range(TT):
                    for q0 in range(0, DCOLS, 512):
                        qw = min(512, DCOLS - q0)
                        nc.tensor.matmul(
                            px[:, q0:q0 + qw],
                            x_bf[:, tt, ko * P:(ko + 1) * P],
                            D_sb[:, tt, q0:q0 + qw],
                            start=(tt == 0), stop=(tt == TT - 1))
                cp(ko, xeT[:, ko, :], px[:, :DCOLS])
        # front pool released here

        # deferred sync-queue half of the w13 stream (emitted after the front
        # section so the x/const loads on the sync queue run first)
        for (wg, j, co, ncols) in w13_sync:
            nc.sync.dma_start(wg.rearrange("p k f -> p (k f)")[:, :ncols],
                              w13_d.ap()[j, :, co:co + ncols])


        # ---- phase A: all slots w13 -> act -> actT ----
        with tc.tile_pool(name="expA", bufs=2) as ea:
            for j in range(4):
                ct = cts[j]
                act = ea.tile([P, 2, I], b16, tag="act", name="act")
                for fci, (fo, fw) in enumerate(FCH):
                    pgus = [mmw(f"pgu{ci}") for ci in range(ct)]
                    for kg in range(HT // KG):
                        wg = next(w13_tiles)
                        for kl in range(KG):
                            ko = kg * KG + kl
                            for ci in range(ct):
                                lhs = xeT[:, ko,
                                          offs[j] + ci * P: offs[j] + (ci + 1) * P]
                                for q0 in range(0, 2 * fw, 512):
                                    qw = min(512, 2 * fw - q0)
                                    nc.tensor.matmul(
                                        pgus[ci][:, q0:q0 + qw], lhs,
                                        wg.rearrange("p k f -> p (k f)")
                                        [:, kl * 2 * fw + q0:kl * 2 * fw + q0 + qw],
                                        start=(ko == 0),
                                        stop=(ko == HT - 1))
                    for ci in range(ct):
                        tmpa = sp.tile([P, 512], b16, tag="tmpa")
                        nc.scalar.activation(tmpa[:, :fw], pgus[ci][:, :fw],
                                             Act.Silu)
                        nc.vector.tensor_tensor(act[:, ci, fo:fo + fw],
                                                tmpa[:, :fw],
                                                pgus[ci][:, fw:2 * fw],
                                                Alu.mult)
                # transpose act -> actT [i, ctile]
                for ci in range(ct):
                    for io in range(IT):
                        pt4 = psA.tile([P, P], b16, tag="sm", name="pt4")
                        nc.tensor.transpose(pt4[:], act[:, ci, io * P:(io + 1) * P],
                                            id16_sb[:])
                        cp(io, actT[:, io, cbase[j] + ci, :], pt4[:])

        w13ctx.close()  # release w13 stream space before phase B

        # ---- phase B: w2 per h-half, combine, overlapped ReduceScatter ----
        with tc.tile_pool(name="phB", bufs=3) as pb_:
            yes = [pb_.tile([P, 1024], b16, tag=f"ye{cb}", name=f"ye{cb}",
                            bufs=1) for cb in range(NCT)]
            # preload shared-down weights for BOTH h-halves before any
            # collective is issued (a DMA behind an in-flight collective can
            # stall on some queues).
            wdn_sb = pb_.tile([P, 2, 3, 1024], b16, tag="wdn", bufs=1)
            nc.scalar.dma_start(wdn_sb[:, 0], wdn_d.ap()[0])
            nc.scalar.dma_start(wdn_sb[:, 1], wdn_d.ap()[1])
            for hh in range(2):
                for j in range(4):
                    ct = cts[j]
                    pys = [mmw(f"py{ci}") for ci in range(ct)]
                    for kgi, (ko0, kn) in enumerate(KOG):
                        par = (hh * 4 + j + kgi) % 2
                        w2c = pb_.tile([P, 3, 1024], b16,
                                       tag="w2ca" if par == 0 else "w2cb",
                                       name="w2c", bufs=3)
                        eng2 = nc.sync if par == 0 else nc.scalar
                        eng2.dma_start(w2c[:, :kn, :],
                                       w2_d.ap()[hh, j, :, ko0:ko0 + kn, :])
                        for kl in range(kn):
                            ko = ko0 + kl
                            for ci in range(ct):
                                for q0 in (0, 512):
                                    nc.tensor.matmul(
                                        pys[ci][:, q0:q0 + 512],
                                        actT[:, ko, cbase[j] + ci, :],
                                        w2c[:, kl, q0:q0 + 512],
                                        start=(ko == 0), stop=(ko == IT - 1))
                    for ci in range(ct):
                        nc.vector.tensor_copy(out=yes[cbase[j] + ci][:],
                                              in_=pys[ci][:])

                # combine: routed ctiles + shared slice -> bf16 partial
                for tt in range(TT):
                    po = mmw("po")
                    for q0 in (0, 512):
                        for q, cb in enumerate(range(NCT)):
                            nc.tensor.matmul(
                                po[:, q0:q0 + 512],
                                WcT[:, cb, tt * P:(tt + 1) * P],
                                yes[cb][:, q0:q0 + 512],
                                start=(q == 0), stop=False)
                        for io in range(3):
                            nc.tensor.matmul(
                                po[:, q0:q0 + 512],
                                actShT[:, io, tt * P:(tt + 1) * P],
                                wdn_sb[:, hh, io, q0:q0 + 512],
                                start=False, stop=(io == 2))
                    if single_core:
                        stg32 = op_.tile([P, 1024], f32, tag="stg32")
                        nc.vector.tensor_copy(out=stg32[:], in_=po[:])
                        nc.sync.dma_start(
                            out_d.ap()[tt * P:(tt + 1) * P,
                                       hh * 1024:(hh + 1) * 1024], stg32[:])
                    else:
                        stg = op_.tile([P, 1024], b16, tag="stg")
                        nc.vector.tensor_copy(out=stg[:], in_=po[:])
                        nc.sync.dma_start(
                            partial_d[hh].ap()[tt * P:(tt + 1) * P, :], stg[:])

                if not single_core:
                    nc.gpsimd.collective_compute(
                        "ReduceScatter", Alu.add,
                        replica_groups=[list(range(NCORES))],
                        ins=[partial_d[hh].ap().opt()],
                        outs=[rs_d[hh].ap().opt()],
                    )

            # epilogue: rs (bf16) -> fp32 out slice
            if not single_core:
                for hh in range(2):
                    rs_sb = op_.tile([T // NCORES, 1024], b16, tag="rs_sb")
                    nc.sync.dma_start(rs_sb[:], rs_d[hh].ap())
                    rs_f = op_.tile([T // NCORES, 1024], f32, tag="rs_f")
                    # gpsimd (not DVE): the scheduler may hoist this RS-gated
                    # cast ahead of phase-B copies on the chosen engine's
                    # stream, which would serialize the hh=1 combine behind
                    # the first ReduceScatter.  gpsimd is already idle /
                    # RS-ordered here.
                    nc.gpsimd.tensor_copy(out=rs_f[:], in_=rs_sb[:])
                    nc.sync.dma_start(
                        out_d.ap()[:, hh * 1024:(hh + 1) * 1024], rs_f[:])

    nc.compile()
    return nc


_NC_CACHE = {}


def _pack_inputs(x, gate_w, bias, w13, w2, sgu, sdn, groups, slot_caps):
    """Per-core in_maps with DMA-friendly packed weight layouts."""
    CAPMAX = max(slot_caps)
    iota = np.tile(np.arange(CAPMAX, dtype=np.float32), (P, 1))
    triu = np.triu(np.ones((P, P), np.float32), 1).astype(bf16)
    ones = np.ones((P, P), bf16)
    id32 = np.eye(P, dtype=np.float32)
    id16 = np.eye(P, dtype=np.float32).astype(bf16)
    bias_b = np.tile(bias[None, :], (P, 1)).astype(np.float32)

    def pack_w13(w):   # w: [H, 2I] fp32 -> packed [P, W13C] bf16
        blocks = []
        for fo, fw in FCH:
            for kg in range(HT // KG):
                for kl in range(KG):
                    ko = kg * KG + kl
                    rows = slice(ko * P, (ko + 1) * P)
                    blocks.append(np.concatenate(
                        [w[rows, fo:fo + fw], w[rows, I + fo:I + fo + fw]],
                        axis=1))
        return np.concatenate(blocks, axis=1).astype(bf16)

    def pack_w2(w):    # w: [I, H] fp32 -> [2, P, IT, 1024] bf16
        r = w.reshape(IT, P, H).transpose(1, 0, 2)  # [P, IT, H]
        return np.stack([r[:, :, 0:1024], r[:, :, 1024:2048]]).astype(bf16)

    in_maps = []
    for core in range(NCORES):
        sel = np.zeros((E, 4), np.float32)
        for j, e in enumerate(groups[core]):
            sel[e, j] = 1.0
        # shared slices, padded to 384
        gate = np.zeros((H, ISHP), np.float32)
        up = np.zeros((H, ISHP), np.float32)
        gate[:, :ISH] = sgu[:, core * ISH:(core + 1) * ISH]
        up[:, :ISH] = sgu[:, 2 * I + core * ISH:2 * I + (core + 1) * ISH]
        wgup = np.zeros((3, P, HT, 256), np.float32)
        for w in range(3):
            pairc = np.concatenate(
                [gate[:, w * P:(w + 1) * P], up[:, w * P:(w + 1) * P]], axis=1)
            wgup[w] = pairc.reshape(HT, P, 256).transpose(1, 0, 2)
        dn = np.zeros((ISHP, H), np.float32)
        dn[:ISH] = sdn[core * ISH:(core + 1) * ISH, :]
        dnr = dn.reshape(3, P, H).transpose(1, 0, 2)  # [P, 3, H]
        wdnp = np.stack([dnr[:, :, 0:1024], dnr[:, :, 1024:2048]])

        in_maps.append({
            "x": x, "gate_w": gate_w, "bias_b": bias_b,
            "w13p": np.stack([pack_w13(w13[e]) for e in groups[core]]),
            "w2p": np.stack([pack_w2(w2[e]) for e in groups[core]], axis=1),
            "wgup": np.ascontiguousarray(wgup.astype(bf16)),
            "wdnp": np.ascontiguousarray(wdnp.astype(bf16)),
            "sel": sel, "iota_r": iota, "triu_b": triu, "ones_b": ones,
            "id_f32": id32, "id_b16": id16,
        })
    return in_maps


def kernel(hidden_states, residual, gate_w, bias, w13, w2, shared_gate_up,
           shared_down):
    from concourse.bass_utils import run_bass_kernel_spmd

    x = np.ascontiguousarray(np.asarray(hidden_states, np.float32))
    gate_w = np.ascontiguousarray(np.asarray(gate_w, np.float32))
    bias = np.asarray(bias, np.float32)
    w13 = np.asarray(w13, np.float32)
    w2 = np.asarray(w2, np.float32)
    sgu = np.asarray(shared_gate_up, np.float32)
    sdn = np.asarray(shared_down, np.float32)

    loads = _host_loads(x, gate_w, bias)
    groups, slot_caps = _plan_slots(loads)

    key = tuple(slot_caps)
    if key not in _NC_CACHE:
        _NC_CACHE[key] = _build_nc(slot_caps)
    nc = _NC_CACHE[key]

    in_maps = _pack_inputs(x, gate_w, bias, w13, w2, sgu, sdn, groups,
                           slot_caps)
    res = run_bass_kernel_spmd(nc, in_maps, core_ids=list(range(NCORES)))
    out = np.concatenate([res.results[c]["out_slice"] for c in range(NCORES)],
                         axis=0)
    return out.astype(np.float32)


# revision 12
# speedup vs baseline: 1.0321x; 1.0119x over previous
"""DeepSeek-MoE Trainium2 kernel (8 NeuronCores, expert-parallel), v2.

Strategy
--------
* Routing (sigmoid + grouped top-k, DeepSeek noaux_tc) is replicated on every
  core in fp32 (top-k margins in this regime are ~2e-5, so bf16 routing would
  flip expert selections).
* Dispatch/combine are dense one-hot matmuls built on-device from the routing
  result: rank-within-expert comes from an exclusive cumsum over tokens
  realized as a matmul with triangular/ones masks, and the one-hot dispatch
  matrix D[t, c] = (rank[t, e_slot] == c) is built with per-partition
  tensor_scalar(is_equal) against an iota row.
* Expert parallelism: 4 experts per core (load-balanced bin-packing computed
  on the host at call time from the actual routing), per-slot capacities are
  compile-time (multiples of 128 covering the observed loads + margin).
* Expert weights are downcast to bf16 on the host and repacked so every DMA
  reads long contiguous per-partition lines.  A deep prefetch pool on the
  gpsimd DMA queue streams w13 from t=0.
* Schedule: phase A runs all four experts' w13 GEMM + silu*up + transpose
  (actT for all capacity tiles stays in SBUF).  Phase B streams w2 one
  h-half at a time, combines routed+shared into a bf16 partial and launches
  the ReduceScatter for half 0 while half 1 is still computing.
* Shared experts are sharded over their intermediate dim (352 channels/core),
  computed in [f, t] orientation (pair-waves) so no transpose is needed.
* Cross-core combine: bf16 ReduceScatter(add) per h-half; core r returns
  tokens [64r, 64r+64) and the host concatenates the 8 slices.
"""

import numpy as np
import ml_dtypes

T, H, E, K, I = 512, 2048, 32, 8, 1408
NG, TKG = 8, 4
RSF = 2.5
NCORES = 8
P = 128
ISH = 2 * I // NCORES  # 352: shared-expert intermediate slice per core
ISHP = 384             # padded to 3*128
HT = H // P            # 16 h-tiles
TT = T // P            # 4 token tiles
IT = I // P            # 11 i-tiles
GS = E // NG           # 4 experts per group
BIG = 1.0e9

bf16 = ml_dtypes.bfloat16

# w13 f-chunks (gate|up pairs of width fw packed adjacently)
FCH = []
_fo = 0
while _fo < I:
    FCH.append((_fo, min(512, I - _fo)))
    _fo += 512
KG = 4  # ko-tiles per w13 DMA chunk
# column offset of chunk (fci, kg) in the packed w13 stream
_W13OFF = {}
_off = 0
for _fci, (_fo, _fw) in enumerate(FCH):
    for _kg in range(HT // KG):
        _W13OFF[(_fci, _kg)] = _off
        _off += KG * 2 * _fw
W13C = _off  # 45056
KOG = [(0, 3), (3, 3), (6, 3), (9, 2)]  # w2 ko groups


# ----------------------------------------------------------------------------
# Host-side routing mirror (only used to pick expert->core assignment and
# compile-time slot capacities; the device re-computes routing exactly).
# ----------------------------------------------------------------------------
def _host_loads(x, gate_w, bias):
    logits = (x.astype(np.float32) @ gate_w.astype(np.float32)).astype(np.float32)
    scores = (1.0 / (1.0 + np.exp(-logits))).astype(np.float32)
    sb = scores + bias[None, :].astype(np.float32)
    g = sb.reshape(T, NG, GS)
    pair = [g[..., i] + g[..., j] for i in range(GS) for j in range(i + 1, GS)]
    grp = np.max(np.stack(pair, -1), -1)
    gmask = np.zeros((T, NG), np.float32)
    gw = grp.copy()
    for _ in range(TKG):
        mx = gw.max(-1, keepdims=True)
        eq = (gw == mx).astype(np.float32)
        gmask += eq
        gw -= eq * BIG
    emask = np.repeat(gmask, GS, axis=1)
    m = sb + (emask * BIG - BIG)
    kmask = np.zeros((T, E), np.float32)
    for _ in range(K):
        mx = m.max(-1, keepdims=True)
        eq = (m == mx).astype(np.float32)
        kmask += eq
        m -= eq * BIG
    return kmask.sum(0)


def _plan_slots(loads, margin=2):
    caps = (np.ceil((loads + margin) / P).astype(int) * P).clip(P, None)
    order = np.argsort(-(caps * 1000 + loads))
    groups = [[] for _ in range(NCORES)]
    gsum = [0] * NCORES
    for e in order:
        cand = [i for i in sorted(range(NCORES), key=lambda i: (gsum[i], len(groups[i])))
                if len(groups[i]) < 4]
        i = cand[0]
        groups[i].append(int(e))
        gsum[i] += caps[e]
    for i in range(NCORES):
        groups[i].sort(key=lambda e: -caps[e])
    slot_caps = [int(max(caps[groups[i][j]] for i in range(NCORES))) for j in range(4)]
    return groups, slot_caps


# ----------------------------------------------------------------------------
# Device program
# ----------------------------------------------------------------------------
def _build_nc(slot_caps, single_core=False):
    import concourse.mybir as mybir
    import concourse.tile as tile
    from concourse import bacc
    from contextlib import ExitStack

    f32 = mybir.dt.float32
    b16 = mybir.dt.bfloat16
    Alu = mybir.AluOpType
    Act = mybir.ActivationFunctionType
    Ax = mybir.AxisListType

    cts = [c // P for c in slot_caps]            # ctiles per slot
    offs = np.cumsum([0] + slot_caps).tolist()   # D column offsets
    DCOLS = offs[-1]
    NCT = sum(cts)                               # total ctiles on this core
    cbase = np.cumsum([0] + cts).tolist()        # global ctile index base per slot
    CAPMAX = max(slot_caps)

    nc = bacc.Bacc("TRN2", target_bir_lowering=False, debug=False,
                   num_devices=1 if single_core else NCORES)

    # ---- I/O ----
    x_d = nc.dram_tensor("x", [T, H], f32, kind="ExternalInput")
    gw_d = nc.dram_tensor("gate_w", [H, E], f32, kind="ExternalInput")
    bias_d = nc.dram_tensor("bias_b", [P, E], f32, kind="ExternalInput")
    w13_d = nc.dram_tensor("w13p", [4, P, W13C], b16, kind="ExternalInput")
    w2_d = nc.dram_tensor("w2p", [2, 4, P, IT, 1024], b16, kind="ExternalInput")
    wgu_d = nc.dram_tensor("wgup", [3, P, HT, 256], b16, kind="ExternalInput")
    wdn_d = nc.dram_tensor("wdnp", [2, P, 3, 1024], b16, kind="ExternalInput")
    sel_d = nc.dram_tensor("sel", [E, 4], f32, kind="ExternalInput")
    iota_d = nc.dram_tensor("iota_r", [P, CAPMAX], f32, kind="ExternalInput")
    triu_d = nc.dram_tensor("triu_b", [P, P], b16, kind="ExternalInput")
    ones_d = nc.dram_tensor("ones_b", [P, P], b16, kind="ExternalInput")
    id32_d = nc.dram_tensor("id_f32", [P, P], f32, kind="ExternalInput")
    id16_d = nc.dram_tensor("id_b16", [P, P], b16, kind="ExternalInput")
    out_d = nc.dram_tensor("out_slice",
                           [T, H] if single_core else [T // NCORES, H], f32,
                           kind="ExternalOutput")

    partial_d = [nc.dram_tensor(f"partial{i}", [T, 1024], b16,
                                kind="Internal") for i in range(2)]
    rs_d = [nc.dram_tensor(f"rs_out{i}", [T // NCORES, 1024], b16,
                           kind="Internal") for i in range(2)]

    def cp(i, out, in_):
        # alternate psum/sbuf copies between DVE and ACT to balance engines
        if i % 2 == 0:
            nc.vector.tensor_copy(out=out, in_=in_)
        else:
            nc.scalar.copy(out, in_)

    xr = x_d.ap().rearrange("(tt p) h -> p tt h", p=P)
    gwr = gw_d.ap().rearrange("(ko p) e -> p ko e", p=P)

    with tile.TileContext(nc) as tc, ExitStack() as ctx:
        pc = ctx.enter_context(tc.tile_pool(name="persist", bufs=1))
        sp = ctx.enter_context(tc.tile_pool(name="smalls", bufs=2))
        psA = ctx.enter_context(tc.tile_pool(name="psumA", bufs=2, space="PSUM"))
        psB = ctx.enter_context(tc.tile_pool(name="psumB", bufs=1, space="PSUM"))
        op_ = ctx.enter_context(tc.tile_pool(name="ostage", bufs=2))
        # deep w13 prefetch stream: opened early so its DMAs start at t=0;
        # closed after phase A so phase B reuses the space
        w13ctx = ExitStack()
        w13p_pool = w13ctx.enter_context(tc.tile_pool(name="w13s", bufs=3))

        pctr = 0

        def mmw(name):
            # three rotating 2-bank wide accumulators
            nonlocal pctr
            pctr += 1
            return psB.tile([P, 1024], f32, tag=f"mmw{pctr % 3}", name=name)

        # ---- w13 stream split across two DMA queues ----
        # gpsimd half issues at t=0 (prefetch); sync half is emitted after the
        # front section so it cannot block the x / const loads on that queue.
        w13_tiles = []   # consumption order
        w13_sync = []    # (tile, j, co, ncols) deferred to the sync queue
        nchunk = 0
        for j in range(4):
            for fci, (fo, fw) in enumerate(FCH):
                for kg in range(HT // KG):
                    tg = "wga" if nchunk % 2 == 0 else "wgb"
                    wg = w13p_pool.tile([P, KG, 1024], b16, tag=tg, name="wg",
                                        bufs=3)
                    co = _W13OFF[(fci, kg)]
                    if nchunk % 2 == 0:
                        nc.gpsimd.dma_start(
                            wg.rearrange("p k f -> p (k f)")[:, :KG * 2 * fw],
                            w13_d.ap()[j, :, co:co + KG * 2 * fw])
                    else:
                        w13_sync.append((wg, j, co, KG * 2 * fw))
                    w13_tiles.append(wg)
                    nchunk += 1
        w13_tiles = iter(w13_tiles)

        # ---- constants needed first on the critical path ----
        id32_sb = pc.tile([P, P], f32, tag="id32")
        nc.sync.dma_start(id32_sb[:], id32_d.ap())
        gw_sb = pc.tile([P, HT, E], f32, tag="gw")
        nc.sync.dma_start(gw_sb[:], gwr)

        # persistent activation/data tiles
        lg_sb = pc.tile([P, TT, E], f32, tag="lg")
        actShT = pc.tile([P, 3, T], b16, tag="actShT")
        xeT = pc.tile([P, HT, DCOLS], b16, tag="xeT")
        actT = pc.tile([P, IT, NCT, P], b16, tag="actT")

        with tc.tile_pool(name="front", bufs=2) as fp:
            x_bf = fp.tile([P, TT, H], b16, tag="xb", bufs=1)
            xT_bf = fp.tile([P, HT, T], b16, tag="xTb", bufs=1)
            # ---- stream x in 512-col chunks: cast bf16, x^T (PE), logits ----
            for hc in range(4):
                xf = fp.tile([P, TT, 512], f32, tag="xf")
                nc.sync.dma_start(xf[:], xr[:, :, hc * 512:(hc + 1) * 512])
                cp(hc, x_bf[:, :, hc * 512:(hc + 1) * 512], xf[:])
                xtf = fp.tile([P, 4, T], f32, tag="xtf")  # [hp, ho_local, t]
                for hl in range(4):
                    for tt in range(TT):
                        pt = psA.tile([P, P], f32, tag="sm", name="pt_x")
                        nc.tensor.transpose(pt[:], xf[:, tt, hl * P:(hl + 1) * P],
                                            id32_sb[:])
                        cp(tt, xtf[:, hl, tt * P:(tt + 1) * P], pt[:])
                    cp(hl, xT_bf[:, hc * 4 + hl, :], xtf[:, hl, :])
                for tt in range(TT):
                    pl = psA.tile([P, E], f32, tag="sm", name="pl")
                    for hl in range(4):
                        nc.tensor.matmul(pl[:], xtf[:, hl, tt * P:(tt + 1) * P],
                                         gw_sb[:, hc * 4 + hl, :],
                                         start=(hl == 0), stop=(hl == 3))
                    if hc == 0:
                        nc.vector.tensor_copy(out=lg_sb[:, tt, :], in_=pl[:])
                    else:
                        nc.vector.tensor_tensor(lg_sb[:, tt, :], lg_sb[:, tt, :],
                                                pl[:], Alu.add)

            # ---- remaining constants (scalar queue; sync stays on x) ----
            bias_sb = pc.tile([P, E], f32, tag="bias")
            nc.scalar.dma_start(bias_sb[:], bias_d.ap())
            sel_sb = pc.tile([E, 4], f32, tag="sel")
            nc.scalar.dma_start(sel_sb[:], sel_d.ap())
            iota_sb = pc.tile([P, CAPMAX], f32, tag="iota")
            nc.scalar.dma_start(iota_sb[:], iota_d.ap())
            triu_sb = pc.tile([P, P], b16, tag="triu")
            nc.scalar.dma_start(triu_sb[:], triu_d.ap())
            ones_sb = pc.tile([P, P], b16, tag="ones")
            nc.scalar.dma_start(ones_sb[:], ones_d.ap())
            id16_sb = pc.tile([P, P], b16, tag="id16")
            nc.scalar.dma_start(id16_sb[:], id16_d.ap())

            # ---- shared expert gate/up in [f, t] orientation (pair-waves) ----
            # wave w covers gate f-cols [128w,128w+128) and up f-cols likewise
            # (padded to 384); psum [f, t] holds gate|up halves in one tile.
            for w in range(3):
                psh = mmw(f"psh{w}")  # [P, 1024]: cols 0:512 gate, 512:1024 up
                for kg in range(HT // KG):
                    wguc = fp.tile([P, KG, 256], b16, tag="wguc")
                    nc.scalar.dma_start(wguc[:],
                                        wgu_d.ap()[w, :, kg * KG:(kg + 1) * KG, :])
                    for kl in range(KG):
                        ko = kg * KG + kl
                        nc.tensor.matmul(psh[:, 0:512], wguc[:, kl, 0:P],
                                         xT_bf[:, ko, :],
                                         start=(ko == 0), stop=(ko == HT - 1))
                        nc.tensor.matmul(psh[:, 512:1024], wguc[:, kl, P:256],
                                         xT_bf[:, ko, :],
                                         start=(ko == 0), stop=(ko == HT - 1))
                tmpsh = sp.tile([P, 512], b16, tag="tmpsh")
                nc.scalar.activation(tmpsh[:], psh[:, 0:512], Act.Silu)
                nc.vector.tensor_tensor(actShT[:, w, :], tmpsh[:],
                                        psh[:, 512:1024], Alu.mult)

            # ---- routing (fp32, on [P, TT, NG, GS] layouts) ----
            scores = pc.tile([P, TT, NG, GS], f32, tag="scores")
            nc.scalar.activation(scores.rearrange("p t g s -> p t (g s)"), lg_sb[:],
                                 Act.Sigmoid)
            sbb = pc.tile([P, TT, NG, GS], f32, tag="sbb")
            nc.vector.tensor_tensor(
                sbb[:], scores[:],
                bias_sb.rearrange("p (g s) -> p g s", g=NG)[:, None, :, :]
                .to_broadcast([P, TT, NG, GS]), Alu.add)

            grp = sp.tile([P, TT, NG], f32, tag="grp")
            pw = sp.tile([P, TT, NG], f32, tag="pw")
            first = True
            for i in range(GS):
                for j in range(i + 1, GS):
                    dst = grp if first else pw
                    nc.vector.tensor_tensor(dst[:], sbb[:, :, :, i], sbb[:, :, :, j],
                                            Alu.add)
                    if not first:
                        nc.vector.tensor_tensor(grp[:], grp[:], pw[:], Alu.max)
                    first = False

            gmask = sp.tile([P, TT, NG], f32, tag="gmask")
            tmpg = sp.tile([P, TT, NG], f32, tag="tmpg")
            mxg = sp.tile([P, TT], f32, tag="mxg")
            for r in range(TKG):
                nc.vector.reduce_max(mxg[:], grp[:], axis=Ax.X)
                nc.vector.tensor_tensor(tmpg[:], grp[:],
                                        mxg[:, :, None].to_broadcast([P, TT, NG]),
                                        Alu.is_equal)
                if r == 0:
                    nc.vector.tensor_copy(out=gmask[:], in_=tmpg[:])
                else:
                    nc.vector.tensor_tensor(gmask[:], gmask[:], tmpg[:], Alu.add)
                if r < TKG - 1:
                    nc.vector.tensor_scalar(tmpg[:], tmpg[:], BIG, None, Alu.mult)
                    nc.vector.tensor_tensor(grp[:], grp[:], tmpg[:], Alu.subtract)

            m_t = pc.tile([P, TT, NG, GS], f32, tag="mt")
            nc.vector.tensor_scalar(m_t[:], gmask[:, :, :, None]
                                    .to_broadcast([P, TT, NG, GS]),
                                    BIG, -BIG, Alu.mult, Alu.add)
            nc.vector.tensor_tensor(m_t[:], m_t[:], sbb[:], Alu.add)
            m_f = m_t.rearrange("p t g s -> p t (g s)")

            kmask = pc.tile([P, TT, E], f32, tag="kmask")
            tmpk = sp.tile([P, TT, E], f32, tag="tmpk")
            mxk = sp.tile([P, TT], f32, tag="mxk")
            for r in range(K):
                nc.vector.reduce_max(mxk[:], m_f, axis=Ax.X)
                nc.vector.tensor_tensor(tmpk[:], m_f,
                                        mxk[:, :, None].to_broadcast([P, TT, E]),
                                        Alu.is_equal)
                if r == 0:
                    nc.vector.tensor_copy(out=kmask[:], in_=tmpk[:])
                else:
                    nc.vector.tensor_tensor(kmask[:], kmask[:], tmpk[:], Alu.add)
                if r < K - 1:
                    nc.vector.tensor_scalar(tmpk[:], tmpk[:], BIG, None, Alu.mult)
                    nc.vector.tensor_tensor(m_f, m_f, tmpk[:], Alu.subtract)

            wsel = sp.tile([P, TT, E], f32, tag="wsel")
            nc.vector.tensor_tensor(wsel[:], kmask[:],
                                    scores.rearrange("p t g s -> p t (g s)"),
                                    Alu.mult)
            denom = sp.tile([P, TT], f32, tag="denom")
            nc.vector.reduce_sum(denom[:], wsel[:], axis=Ax.X)
            winv = sp.tile([P, TT], f32, tag="winv")
            nc.vector.reciprocal(winv[:], denom[:])
            nc.vector.tensor_scalar(winv[:], winv[:], RSF, None, Alu.mult)
            W_t = pc.tile([P, TT, E], f32, tag="Wt")
            nc.vector.tensor_tensor(W_t[:], wsel[:],
                                    winv[:, :, None].to_broadcast([P, TT, E]),
                                    Alu.mult)

            count_bf = sp.tile([P, TT, E], b16, tag="countb")
            nc.scalar.copy(count_bf[:], kmask[:])
            baseA = pc.tile([P, TT, E], f32, tag="baseA")
            namask = sp.tile([P, TT, E], f32, tag="namask")
            nc.vector.tensor_scalar(namask[:], kmask[:], -1.0e6, 1.0e6,
                                    Alu.mult, Alu.add)
            for mt in range(TT):
                pb = psA.tile([P, E], f32, tag="sm", name="pb")
                for kk in range(mt + 1):
                    lhs = ones_sb if kk < mt else triu_sb
                    nc.tensor.matmul(pb[:], lhs[:], count_bf[:, kk, :],
                                     start=(kk == 0), stop=(kk == mt))
                nc.vector.tensor_tensor(baseA[:, mt, :], pb[:], namask[:, mt, :],
                                        Alu.add)

            # transpose baseA, W -> [E, t]; select this core's 4 experts via sel
            baT = pc.tile([E, TT, P], f32, tag="baT")
            wT = pc.tile([E, TT, P], f32, tag="wT")
            for tt in range(TT):
                pt1 = psA.tile([E, P], f32, tag="sm", name="pt1")
                nc.tensor.transpose(pt1[:], baseA[:, tt, :], id32_sb[:])
                nc.vector.tensor_copy(out=baT[:, tt, :], in_=pt1[:])
                pt2 = psA.tile([E, P], f32, tag="sm", name="pt2")
                nc.tensor.transpose(pt2[:], W_t[:, tt, :], id32_sb[:])
                nc.scalar.copy(wT[:, tt, :], pt2[:])
            bsel = pc.tile([P, TT, 4], f32, tag="bsel")
            wsel4 = pc.tile([P, TT, 4], f32, tag="wsel4")
            for tt in range(TT):
                pb4 = psA.tile([P, 4], f32, tag="sm", name="pb4")
                nc.tensor.matmul(pb4[:], baT[:, tt, :], sel_sb[:], start=True,
                                 stop=True)
                nc.vector.tensor_copy(out=bsel[:, tt, :], in_=pb4[:])
                pw4 = psA.tile([P, 4], f32, tag="sm", name="pw4")
                nc.tensor.matmul(pw4[:], wT[:, tt, :], sel_sb[:], start=True,
                                 stop=True)
                nc.scalar.copy(wsel4[:, tt, :], pw4[:])

            # dispatch one-hot D (bf16); combine weights Wc (bf16) -> WcT
            D_sb = pc.tile([P, TT, DCOLS], b16, tag="D")
            WcT = pc.tile([P, NCT, T], b16, tag="WcT")
            for tt in range(TT):
                for j in range(4):
                    cap = slot_caps[j]
                    nc.vector.tensor_scalar(D_sb[:, tt, offs[j]:offs[j] + cap],
                                            iota_sb[:, :cap], bsel[:, tt, j:j + 1],
                                            None, Alu.is_equal)
                    wcs = sp.tile([P, 256], b16, tag="wcs")
                    nc.vector.tensor_scalar(wcs[:, :cap], iota_sb[:, :cap],
                                            bsel[:, tt, j:j + 1],
                                            wsel4[:, tt, j:j + 1],
                                            Alu.is_equal, Alu.mult)
                    for cl in range(cts[j]):
                        ptw = psA.tile([P, P], b16, tag="sm", name="ptw")
                        nc.tensor.transpose(ptw[:], wcs[:, cl * P:(cl + 1) * P],
                                            id16_sb[:])
                        cp(cl + tt, WcT[:, cbase[j] + cl, tt * P:(tt + 1) * P],
                           ptw[:])

            # ---- dispatch matmul: xeT[h, c] = sum_t x[t,h] D[t,c] ----
            for ko in range(HT):
                px = mmw("px")
                for tt in range(TT):
                    for q0 in range(0, DCOLS, 512):
                        qw = min(512, DCOLS - q0)
                        nc.tensor.matmul(
                            px[:, q0:q0 + qw],
                            x_bf[:, tt, ko * P:(ko + 1) * P],
                            D_sb[:, tt, q0:q0 + qw],
                            start=(tt == 0), stop=(tt == TT - 1))
                cp(ko, xeT[:, ko, :], px[:, :DCOLS])
        # front pool released here

        # deferred sync-queue half of the w13 stream (emitted after the front
        # section so the x/const loads on the sync queue run first)
        for (wg, j, co, ncols) in w13_sync:
            nc.sync.dma_start(wg.rearrange("p k f -> p (k f)")[:, :ncols],
                              w13_d.ap()[j, :, co:co + ncols])


        # ---- phase A: all slots w13 -> act -> actT ----
        with tc.tile_pool(name="expA", bufs=2) as ea:
            for j in range(4):
                ct = cts[j]
                act = ea.tile([P, 2, I], b16, tag="act", name="act")
                for fci, (fo, fw) in enumerate(FCH):
                    pgus = [mmw(f"pgu{ci}") for ci in range(ct)]
                    for kg in range(HT // KG):
                        wg = next(w13_tiles)
                        for kl in range(KG):
                            ko = kg * KG + kl
                            for ci in range(ct):
                                lhs = xeT[:, ko,
                                          offs[j] + ci * P: offs[j] + (ci + 1) * P]
                                for q0 in range(0, 2 * fw, 512):
                                    qw = min(512, 2 * fw - q0)
                                    nc.tensor.matmul(
                                        pgus[ci][:, q0:q0 + qw], lhs,
                                        wg.rearrange("p k f -> p (k f)")
                                        [:, kl * 2 * fw + q0:kl * 2 * fw + q0 + qw],
                                        start=(ko == 0),
                                        stop=(ko == HT - 1))
                    for ci in range(ct):
                        tmpa = sp.tile([P, 512], b16, tag="tmpa")
                        nc.scalar.activation(tmpa[:, :fw], pgus[ci][:, :fw],
                                             Act.Silu)
                        nc.vector.tensor_tensor(act[:, ci, fo:fo + fw],
                                                tmpa[:, :fw],
                                                pgus[ci][:, fw:2 * fw],
                                                Alu.mult)
                # transpose act -> actT [i, ctile]
                for ci in range(ct):
                    for io in range(IT):
                        pt4 = psA.tile([P, P], b16, tag="sm", name="pt4")
                        nc.tensor.transpose(pt4[:], act[:, ci, io * P:(io + 1) * P],
                                            id16_sb[:])
                        cp(io, actT[:, io, cbase[j] + ci, :], pt4[:])

        w13ctx.close()  # release w13 stream space before phase B

        # ---- phase B: w2 per h-half, combine, overlapped ReduceScatter ----
        with tc.tile_pool(name="phB", bufs=3) as pb_:
            yes = [pb_.tile([P, 1024], b16, tag=f"ye{cb}", name=f"ye{cb}",
                            bufs=1) for cb in range(NCT)]
            # preload shared-down weights for BOTH h-halves before any
            # collective is issued (a DMA behind an in-flight collective can
            # stall on some queues).
            wdn_sb = pb_.tile([P, 2, 3, 1024], b16, tag="wdn", bufs=1)
            nc.scalar.dma_start(wdn_sb[:, 0], wdn_d.ap()[0])
            nc.scalar.dma_start(wdn_sb[:, 1], wdn_d.ap()[1])
            for hh in range(2):
                for j in range(4):
                    ct = cts[j]
                    pys = [mmw(f"py{ci}") for ci in range(ct)]
                    for kgi, (ko0, kn) in enumerate(KOG):
                        par = (hh * 4 + j + kgi) % 2
                        w2c = pb_.tile([P, 3, 1024], b16,
                                       tag="w2ca" if par == 0 else "w2cb",
                                       name="w2c", bufs=5)
                        eng2 = nc.sync if par == 0 else nc.scalar
                        eng2.dma_start(w2c[:, :kn, :],
                                       w2_d.ap()[hh, j, :, ko0:ko0 + kn, :])
                        for kl in range(kn):
                            ko = ko0 + kl
                            for ci in range(ct):
                                for q0 in (0, 512):
                                    nc.tensor.matmul(
                                        pys[ci][:, q0:q0 + 512],
                                        actT[:, ko, cbase[j] + ci, :],
                                        w2c[:, kl, q0:q0 + 512],
                                        start=(ko == 0), stop=(ko == IT - 1))
                    for ci in range(ct):
                        nc.vector.tensor_copy(out=yes[cbase[j] + ci][:],
                                              in_=pys[ci][:])

                # combine: routed ctiles + shared slice -> bf16 partial
                for tt in range(TT):
                    po = mmw("po")
                    for q0 in (0, 512):
                        for q, cb in enumerate(range(NCT)):
                            nc.tensor.matmul(
                                po[:, q0:q0 + 512],
                                WcT[:, cb, tt * P:(tt + 1) * P],
                                yes[cb][:, q0:q0 + 512],
                                start=(q == 0), stop=False)
                        for io in range(3):
                            nc.tensor.matmul(
                                po[:, q0:q0 + 512],
                                actShT[:, io, tt * P:(tt + 1) * P],
                                wdn_sb[:, hh, io, q0:q0 + 512],
                                start=False, stop=(io == 2))
                    if single_core:
                        stg32 = op_.tile([P, 1024], f32, tag="stg32")
                        nc.vector.tensor_copy(out=stg32[:], in_=po[:])
                        nc.sync.dma_start(
                            out_d.ap()[tt * P:(tt + 1) * P,
                                       hh * 1024:(hh + 1) * 1024], stg32[:])
                    else:
                        stg = op_.tile([P, 1024], b16, tag="stg")
                        nc.vector.tensor_copy(out=stg[:], in_=po[:])
                        nc.sync.dma_start(
                            partial_d[hh].ap()[tt * P:(tt + 1) * P, :], stg[:])

                if not single_core:
                    nc.gpsimd.collective_compute(
                        "ReduceScatter", Alu.add,
                        replica_groups=[list(range(NCORES))],
                        ins=[partial_d[hh].ap().opt()],
                        outs=[rs_d[hh].ap().opt()],
                    )

            # epilogue: rs (bf16) -> fp32 out slice
            if not single_core:
                for hh in range(2):
                    rs_sb = op_.tile([T // NCORES, 1024], b16, tag="rs_sb",
                                     bufs=1)
                    nc.sync.dma_start(rs_sb[:], rs_d[hh].ap())
                    rs_f = op_.tile([T // NCORES, 1024], f32, tag="rs_f",
                                    bufs=1)
                    # gpsimd (not DVE): the scheduler may hoist this RS-gated
                    # cast ahead of phase-B copies on the chosen engine's
                    # stream, which would serialize the hh=1 combine behind
                    # the first ReduceScatter.  gpsimd is already idle /
                    # RS-ordered here.
                    nc.gpsimd.tensor_copy(out=rs_f[:], in_=rs_sb[:])
                    nc.sync.dma_start(
                        out_d.ap()[:, hh * 1024:(hh + 1) * 1024], rs_f[:])

    nc.compile()
    return nc


_NC_CACHE = {}


def _pack_inputs(x, gate_w, bias, w13, w2, sgu, sdn, groups, slot_caps):
    """Per-core in_maps with DMA-friendly packed weight layouts."""
    CAPMAX = max(slot_caps)
    iota = np.tile(np.arange(CAPMAX, dtype=np.float32), (P, 1))
    triu = np.triu(np.ones((P, P), np.float32), 1).astype(bf16)
    ones = np.ones((P, P), bf16)
    id32 = np.eye(P, dtype=np.float32)
    id16 = np.eye(P, dtype=np.float32).astype(bf16)
    bias_b = np.tile(bias[None, :], (P, 1)).astype(np.float32)

    def pack_w13(w):   # w: [H, 2I] fp32 -> packed [P, W13C] bf16
        blocks = []
        for fo, fw in FCH:
            for kg in range(HT // KG):
                for kl in range(KG):
                    ko = kg * KG + kl
                    rows = slice(ko * P, (ko + 1) * P)
                    blocks.append(np.concatenate(
                        [w[rows, fo:fo + fw], w[rows, I + fo:I + fo + fw]],
                        axis=1))
        return np.concatenate(blocks, axis=1).astype(bf16)

    def pack_w2(w):    # w: [I, H] fp32 -> [2, P, IT, 1024] bf16
        r = w.reshape(IT, P, H).transpose(1, 0, 2)  # [P, IT, H]
        return np.stack([r[:, :, 0:1024], r[:, :, 1024:2048]]).astype(bf16)

    in_maps = []
    for core in range(NCORES):
        sel = np.zeros((E, 4), np.float32)
        for j, e in enumerate(groups[core]):
            sel[e, j] = 1.0
        # shared slices, padded to 384
        gate = np.zeros((H, ISHP), np.float32)
        up = np.zeros((H, ISHP), np.float32)
        gate[:, :ISH] = sgu[:, core * ISH:(core + 1) * ISH]
        up[:, :ISH] = sgu[:, 2 * I + core * ISH:2 * I + (core + 1) * ISH]
        wgup = np.zeros((3, P, HT, 256), np.float32)
        for w in range(3):
            pairc = np.concatenate(
                [gate[:, w * P:(w + 1) * P], up[:, w * P:(w + 1) * P]], axis=1)
            wgup[w] = pairc.reshape(HT, P, 256).transpose(1, 0, 2)
        dn = np.zeros((ISHP, H), np.float32)
        dn[:ISH] = sdn[core * ISH:(core + 1) * ISH, :]
        dnr = dn.reshape(3, P, H).transpose(1, 0, 2)  # [P, 3, H]
        wdnp = np.stack([dnr[:, :, 0:1024], dnr[:, :, 1024:2048]])

        in_maps.append({
            "x": x, "gate_w": gate_w, "bias_b": bias_b,
            "w13p": np.stack([pack_w13(w13[e]) for e in groups[core]]),
            "w2p": np.stack([pack_w2(w2[e]) for e in groups[core]], axis=1),
            "wgup": np.ascontiguousarray(wgup.astype(bf16)),
            "wdnp": np.ascontiguousarray(wdnp.astype(bf16)),
            "sel": sel, "iota_r": iota, "triu_b": triu, "ones_b": ones,
            "id_f32": id32, "id_b16": id16,
        })
    return in_maps


def kernel(hidden_states, residual, gate_w, bias, w13, w2, shared_gate_up,
           shared_down):
    from concourse.bass_utils import run_bass_kernel_spmd

    x = np.ascontiguousarray(np.asarray(hidden_states, np.float32))
    gate_w = np.ascontiguousarray(np.asarray(gate_w, np.float32))
    bias = np.asarray(bias, np.float32)
    w13 = np.asarray(w13, np.float32)
    w2 = np.asarray(w2, np.float32)
    sgu = np.asarray(shared_gate_up, np.float32)
    sdn = np.asarray(shared_down, np.float32)

    loads = _host_loads(x, gate_w, bias)
    groups, slot_caps = _plan_slots(loads)

    key = tuple(slot_caps)
    if key not in _NC_CACHE:
        _NC_CACHE[key] = _build_nc(slot_caps)
    nc = _NC_CACHE[key]

    in_maps = _pack_inputs(x, gate_w, bias, w13, w2, sgu, sdn, groups,
                           slot_caps)
    res = run_bass_kernel_spmd(nc, in_maps, core_ids=list(range(NCORES)))
    out = np.concatenate([res.results[c]["out_slice"] for c in range(NCORES)],
                         axis=0)
    return out.astype(np.float32)
